# Initial kernel scaffold
#

# Trainium2 Bass kernel for nn_CustomAttention (cosine-sim multi-head attention).
#
# Sharding over 8 cores: core c handles batch b = c//2 and head group
# g = c%2 (8 of 16 heads).  Each core computes its heads' q/k/v projections
# (Megatron column-parallel), cosine-sim attention, and a partial output
# projection (row-parallel over the 512 feature columns it owns).  The host
# sums the two partial outputs per batch and adds out_b.
#
# Layouts on device (per core):
#   qT/kT: (dims=512, seq=1024) as 4 tiles of (128, 1024); dims on partitions
#          so the scores matmul contracts over head_dim on the partition axis.
#   scores are computed transposed, sT[k, q], so softmax's key-sum is a
#   PE matmul contraction; the ones-column appended to v gives the softmax
#   denominator for free (row 64 of the (65, q) p@v output).
#   l2-norm of k and the per-head logit scale fold into the exp() activation
#   scale (per-partition AP); q's norm is applied via a broadcast multiply.
#
# All heavy matmuls run in float32r (fp32 storage, ~1e-3 matmul accuracy at
# bf16 speed); the p@v pair runs in bf16 (probabilities are in [0, 1]).

import math

import numpy as np

import sys

sys.path.insert(0, "/opt/trn_rl_repo")

import concourse.bass as bass
import concourse.tile as tile
from concourse import bacc, mybir
from concourse.bass_utils import run_bass_kernel_spmd
from concourse.masks import make_identity

N = 1024  # sequence length
B = 4  # batch
C = 1024  # channels
H = 16  # total heads
HD = 64  # head dim
G = 512  # dims per core (8 heads)
NT = 4  # (128,1024) tiles of qT/kT per core
CC = 8  # contraction chunks of 128 over C
ST = 8  # seq tiles of 128
QC = 2  # seq chunks of 512
LOGIT_SCALE_MAX = math.log(1.0 / 0.01)

F32 = mybir.dt.float32
F32R = mybir.dt.float32r
BF16 = mybir.dt.bfloat16
AF = mybir.ActivationFunctionType

_CACHED_NC = None
_LAST_IN_MAPS = None


def build_nc():
    nc = bacc.Bacc("TRN2", target_bir_lowering=False)

    # fp32 data; declared float32r where consumed by fp32r matmuls (the PE
    # rounds internally; numpy side is plain float32 bits either way).
    qt_d = nc.declare_dram_parameter("qt", [C, N], F32R, isOutput=False)
    kt_d = nc.declare_dram_parameter("kt", [C, N], F32R, isOutput=False)
    vt_d = nc.declare_dram_parameter("vt", [C, N], F32R, isOutput=False)
    wq_d = nc.declare_dram_parameter("wq", [C, G], F32R, isOutput=False)
    wk_d = nc.declare_dram_parameter("wk", [C, G], F32R, isOutput=False)
    wv_d = nc.declare_dram_parameter("wv", [C, G], F32R, isOutput=False)
    wo_d = nc.declare_dram_parameter("wo", [G, C], F32R, isOutput=False)
    bq_d = nc.declare_dram_parameter("bq", [128, NT], F32, isOutput=False)
    bk_d = nc.declare_dram_parameter("bk", [128, NT], F32, isOutput=False)
    bv_d = nc.declare_dram_parameter("bv", [1, G], F32R, isOutput=False)
    sel8_d = nc.declare_dram_parameter("sel8", [NT, 128, 8], F32R, isOutput=False)
    ones1_d = nc.declare_dram_parameter("ones1", [1, 128], F32R, isOutput=False)
    sel2T_d = nc.declare_dram_parameter("sel2T", [2, 128], F32R, isOutput=False)
    lsinv2_d = nc.declare_dram_parameter("lsinv2", [8, 1], F32, isOutput=False)
    lsbias_d = nc.declare_dram_parameter("lsbias", [128, 8], F32, isOutput=False)
    out_d = nc.declare_dram_parameter("out", [N, C], F32, isOutput=True)
    rsq_dram = nc.dram_tensor("rsq_scratch", [8, N], F32)
    rq_dram = nc.dram_tensor("rq_scratch", [8, N], F32)

    with tile.TileContext(nc) as tc:
        with (
            tc.tile_pool(name="consts", bufs=1) as consts,
            tc.tile_pool(name="wo_p", bufs=1) as wo_p,
            tc.tile_pool(name="w_p", bufs=10) as w_p,
            tc.tile_pool(name="acts", bufs=9) as acts,
            tc.tile_pool(name="big", bufs=1) as big,
            tc.tile_pool(name="sq_p", bufs=2) as sq_p,
            tc.tile_pool(name="stats", bufs=1) as stats,
            tc.tile_pool(name="eT_p", bufs=2) as eT_p,
            tc.tile_pool(name="xu_p", bufs=2) as xu_p,
            tc.tile_pool(name="bc_p", bufs=2) as bc_p,
            tc.tile_pool(name="outs", bufs=3) as outs_p,
        ):
            qt_r = qt_d[:].rearrange("(cc p) n -> cc p n", p=128)
            kt_r = kt_d[:].rearrange("(cc p) n -> cc p n", p=128)
            vt_r = vt_d[:].rearrange("(cc p) n -> cc p n", p=128)
            wq_r = wq_d[:].rearrange("(cc p) g -> cc p g", p=128)
            wk_r = wk_d[:].rearrange("(cc p) g -> cc p g", p=128)
            wv_r = wv_d[:].rearrange("(cc p) g -> cc p g", p=128)

            # q-tensor chunks stream first: nothing sits ahead of them in the
            # DMA queues, so the first projection matmul starts ~2us in.
            pre_q = []
            for cc in range(CC):
                w_sb = w_p.tile([128, G], F32R, tag="w", name=f"wq{cc}")
                nc.sync.dma_start(out=w_sb[:], in_=wq_r[cc])
                a_sb = acts.tile([128, N], F32R, tag="act", name=f"aq{cc}")
                nc.sync.dma_start(out=a_sb[:], in_=qt_r[cc])
                pre_q.append((w_sb, a_sb))

            # ---- constants ----
            sel8 = consts.tile([128, NT, 8], F32R)
            nc.sync.dma_start(out=sel8[:], in_=sel8_d[:].rearrange("t p e -> p t e"))
            ones1 = consts.tile([1, 128], F32R)
            nc.sync.dma_start(out=ones1[:], in_=ones1_d[:])
            sel2T = consts.tile([2, 128], F32R)
            nc.sync.dma_start(out=sel2T[:], in_=sel2T_d[:])
            lsinv2 = consts.tile([8, 1], F32)
            nc.sync.dma_start(out=lsinv2[:], in_=lsinv2_d[:])
            lsbias = consts.tile([128, 8], F32)
            nc.sync.dma_start(out=lsbias[:], in_=lsbias_d[:])
            bq_sb = consts.tile([128, NT], F32)
            nc.sync.dma_start(out=bq_sb[:], in_=bq_d[:])
            bk_sb = consts.tile([128, NT], F32)
            nc.sync.dma_start(out=bk_sb[:], in_=bk_d[:])
            bv_sb = consts.tile([1, G], F32R)
            nc.sync.dma_start(out=bv_sb[:], in_=bv_d[:])
            ident8 = consts.tile([8, 8], F32)
            make_identity(nc, ident8[:])
            # ---- persistent big tiles ----
            qT = [big.tile([128, N], F32R, tag=f"qT{t}", name=f"qT{t}") for t in range(NT)]
            kT = [big.tile([128, N], F32R, tag=f"kT{t}", name=f"kT{t}") for t in range(NT)]
            v_sb = [big.tile([128, 8, HD + 1], F32R, tag=f"v{s}", name=f"v{s}") for s in range(ST)]
            xt = [big.tile([128, N], F32, tag=f"xt{t}", name=f"xt{t}") for t in range(NT)]
            rskT = stats.tile([128, ST, 8], F32)

            with (
                tc.tile_pool(name="pp", bufs=2, space="PSUM") as pp,
                tc.tile_pool(name="pv", bufs=2, space="PSUM") as pv,
                tc.tile_pool(name="pstat", bufs=2, space="PSUM") as pstat,
            ):
                # ======== q/k projections:  xT_t = (w_x^T chunk)^T @ actT ========
                for name, act_r, w_r, dst, b_sb in (
                    ("q", qt_r, wq_r, qT, bq_sb),
                    ("k", kt_r, wk_r, kT, bk_sb),
                ):
                    if name == "q":
                        w_ch = [p[0] for p in pre_q]
                        a_ch = [p[1] for p in pre_q]
                    else:
                        w_ch = []
                        a_ch = []
                        for cc in range(CC):
                            w_sb = w_p.tile([128, G], F32R, tag="w")
                            nc.sync.dma_start(out=w_sb[:], in_=w_r[cc])
                            a_sb = acts.tile([128, N], F32R, tag="act")
                            nc.sync.dma_start(out=a_sb[:], in_=act_r[cc])
                            w_ch.append(w_sb)
                            a_ch.append(a_sb)
                    for t in range(NT):
                        for qc in range(QC):
                            ps = pp.tile([128, 512], F32, tag="proj")
                            for cc in range(CC):
                                nc.tensor.matmul(
                                    ps[:],
                                    w_ch[cc][:, t * 128 : (t + 1) * 128],
                                    a_ch[cc][:, qc * 512 : (qc + 1) * 512],
                                    start=(cc == 0),
                                    stop=(cc == CC - 1),
                                )
                            # psum -> sbuf with per-dim bias add, f32r rounded
                            nc.vector.tensor_scalar_add(
                                out=dst[t][:, qc * 512 : (qc + 1) * 512],
                                in0=ps[:],
                                scalar1=b_sb[:, t : t + 1],
                            )
                        # squares for the ssq matmul
                        sq = sq_p.tile([128, N], F32R, tag="sq")
                        f32view = dst[t][:].bitcast(F32)
                        nc.vector.tensor_mul(out=sq[:], in0=f32view, in1=f32view)
                        # ssq rows accumulate into (8, N) psum via selector
                        if t == 0:
                            ps_ssq = pstat.tile([8, N], F32, tag="ssq", bufs=1)
                        for qc in range(QC):
                            nc.tensor.matmul(
                                ps_ssq[:, qc * 512 : (qc + 1) * 512],
                                sel8[:, t, :],
                                sq[:, qc * 512 : (qc + 1) * 512],
                                start=(t == 0),
                                stop=(t == NT - 1),
                            )
                    if name == "q":
                        # rsq_q = 1/sqrt(ssq)
                        rsq = stats.tile([8, N], F32, tag="rsq_q")
                        nc.scalar.activation(out=rsq[:], in_=ps_ssq[:], func=AF.Sqrt)
                        nc.vector.reciprocal(out=rsq[:], in_=rsq[:])
                        rsq_q = rsq
                    else:
                        # rsk = ls_h / sqrt(ssq)  (scale folds 1/ls^2)
                        rsk = stats.tile([8, N], F32, tag="rsk")
                        nc.scalar.activation(
                            out=rsk[:], in_=ps_ssq[:], func=AF.Sqrt,
                            bias=0.0, scale=lsinv2[:],
                        )
                        nc.vector.reciprocal(out=rsk[:], in_=rsk[:])

                # transpose rsk rows into per-key columns: (8, 128) -> (128, 8)
                for s in range(ST):
                    ps_t = pstat.tile([128, 8], F32, tag="rskT", bufs=2)
                    nc.tensor.transpose(
                        ps_t[:], rsk[:, s * 128 : (s + 1) * 128], ident8[:]
                    )
                    nc.vector.tensor_copy(out=rskT[:, s, :], in_=ps_t[:])

                # q-hat: multiply qT rows by broadcast 1/||q|| (per head)
                nc.sync.dma_start(out=rsq_dram[:], in_=rsq_q[:])
                for t in range(NT):
                    rqb = bc_p.tile([128, N], F32, tag="rqb")
                    for j in range(2):
                        h = 2 * t + j
                        nc.sync.dma_start(
                            out=rqb[j * 64 : (j + 1) * 64, :],
                            in_=rsq_dram[h : h + 1, :].to_broadcast((64, N)),
                        )
                    nc.vector.tensor_mul(
                        out=qT[t][:], in0=qT[t][:].bitcast(F32), in1=rqb[:]
                    )

                # ======== v projection (natural layout) + bias + ones col ========
                v_ch = []
                wv_ch = []
                for cc in range(CC):
                    wv_sb = w_p.tile([128, G], F32R, tag="w")
                    nc.sync.dma_start(out=wv_sb[:], in_=wv_r[cc])
                    va_sb = acts.tile([128, N], F32R, tag="act")
                    nc.sync.dma_start(out=va_sb[:], in_=vt_r[cc])
                    wv_ch.append(wv_sb)
                    v_ch.append(va_sb)
                for s in range(ST):
                    ps = pv.tile([128, G], F32, tag="vproj")
                    for cc in range(CC):
                        nc.tensor.matmul(
                            ps[:],
                            v_ch[cc][:, s * 128 : (s + 1) * 128],
                            wv_ch[cc][:],
                            start=(cc == 0),
                            stop=False,
                        )
                    # bias add via rank-1 matmul: ones(1,128)^T @ bv(1,512)
                    nc.tensor.matmul(ps[:], ones1[:], bv_sb[:], start=False, stop=True)
                    nc.vector.tensor_copy(
                        out=v_sb[s][:, :, 0:HD],
                        in_=ps[:].rearrange("p (h d) -> p h d", h=8),
                    )
                    nc.vector.memset(v_sb[s][:, :, HD].bitcast(F32), 1.0)

                # wo: after all projection inputs, well before the out-proj
                wo_sb = wo_p.tile([128, NT, C], F32R)
                nc.sync.dma_start(
                    out=wo_sb[:], in_=wo_d[:].rearrange("(t p) c -> p t c", p=128)
                )

            # ======== attention ========
            with (
                tc.tile_pool(name="psT", bufs=2, space="PSUM") as psT,
                tc.tile_pool(name="pxa", bufs=2, space="PSUM") as pxa,
            ):
                for t in range(NT):
                    xa = [
                        pxa.tile([65, 512], F32, tag=f"xa{j}{qc}", name=f"xa{j}{qc}", bufs=1)
                        for qc in range(QC)
                        for j in range(2)
                    ]
                    for s in range(ST):
                        for j in range(2):
                            h = 2 * t + j
                            # full-width scores for one key tile (two psum banks;
                            # each matmul writes one bank)
                            sT = psT.tile([128, N], F32, tag=f"sT{j}", bufs=1)
                            for qc in range(QC):
                                nc.tensor.matmul(
                                    sT[:, qc * 512 : (qc + 1) * 512],
                                    kT[t][j * 64 : (j + 1) * 64, s * 128 : (s + 1) * 128],
                                    qT[t][j * 64 : (j + 1) * 64, qc * 512 : (qc + 1) * 512],
                                    start=True,
                                    stop=True,
                                )
                            # one wide exp amortizes ACT's per-op overhead
                            eT = eT_p.tile([128, N], F32R, tag=f"eT{j}")
                            nc.scalar.activation(
                                out=eT[:], in_=sT[:], func=AF.Exp,
                                bias=lsbias[:, h : h + 1],
                                scale=rskT[:, s, h : h + 1],
                            )
                            for qc in range(QC):
                                nc.tensor.matmul(
                                    xa[2 * qc + j][:],
                                    v_sb[s][:, h, :],
                                    eT[:, qc * 512 : (qc + 1) * 512],
                                    start=(s == 0),
                                    stop=(s == ST - 1),
                                )
                    cst = stats.tile([2, N], F32, tag="cst", bufs=2, name=f"cst{t}")
                    for qc in range(QC):
                        for j in range(2):
                            h = 2 * t + j
                            xu = xu_p.tile([65, 512], F32, tag="xu")
                            nc.vector.tensor_copy(out=xu[:], in_=xa[2 * qc + j][:])
                            # softmax denominator row -> cst[j]
                            nc.sync.dma_start(
                                out=cst[j : j + 1, qc * 512 : (qc + 1) * 512],
                                in_=xu[64:65, :],
                            )
                            # numerator -> xt tile (partition shift for j=1)
                            nc.sync.dma_start(
                                out=xt[t][j * 64 : (j + 1) * 64, qc * 512 : (qc + 1) * 512],
                                in_=xu[0:64, :],
                            )

                    # per-tile normalization.  Interior tiles bounce 1/colsum
                    # through DRAM (latency hides under the next tile); the
                    # last tile uses a PE selector-matmul broadcast instead so
                    # the exposed tail stays short.
                    if t < NT - 1:
                        nc.vector.reciprocal(out=cst[:], in_=cst[:])
                        nc.sync.dma_start(
                            out=rq_dram[2 * t : 2 * t + 2, :], in_=cst[:]
                        )
                        rqc = bc_p.tile([128, N], F32, tag="rqb")
                        for j in range(2):
                            h = 2 * t + j
                            nc.sync.dma_start(
                                out=rqc[j * 64 : (j + 1) * 64, :],
                                in_=rq_dram[h : h + 1, :].to_broadcast((64, N)),
                            )
                        nc.vector.tensor_mul(
                            out=xt[t][:].bitcast(F32R), in0=xt[t][:], in1=rqc[:]
                        )
                    else:
                        with nc.allow_low_precision("f32r rounding of 1/colsum"):
                            nc.vector.reciprocal(
                                out=cst[:].bitcast(F32R), in_=cst[:]
                            )
                        for qc in range(QC):
                            rqc_ps = pxa.tile(
                                [128, 512], F32, tag=f"xa{0}{qc}",
                                name=f"rq{t}{qc}", bufs=1,
                            )
                            nc.tensor.matmul(
                                rqc_ps[:],
                                sel2T[:],
                                cst[:].bitcast(F32R)[:, qc * 512 : (qc + 1) * 512],
                                start=True,
                                stop=True,
                            )
                            nc.vector.tensor_mul(
                                out=xt[t][:, qc * 512 : (qc + 1) * 512].bitcast(F32R),
                                in0=xt[t][:, qc * 512 : (qc + 1) * 512],
                                in1=rqc_ps[:],
                            )

            # ======== output projection (partial over this core's 512 dims) ====
            with tc.tile_pool(name="po", bufs=4, space="PSUM") as po:
                for s in range(ST):
                    for coc in range(2):
                        ps = po.tile([128, 512], F32, tag="out")
                        for t in range(NT):
                            nc.tensor.matmul(
                                ps[:],
                                xt[t][:].bitcast(F32R)[:, s * 128 : (s + 1) * 128],
                                wo_sb[:, t, coc * 512 : (coc + 1) * 512],
                                start=(t == 0),
                                stop=(t == NT - 1),
                            )
                        o_sb = outs_p.tile([128, 512], F32, tag="osb")
                        nc.vector.tensor_copy(out=o_sb[:], in_=ps[:])
                        nc.sync.dma_start(
                            out=out_d[:][
                                s * 128 : (s + 1) * 128, coc * 512 : (coc + 1) * 512
                            ],
                            in_=o_sb[:],
                        )

    nc.compile()
    return nc


def kernel(
    query, key, value, in_proj_w, in_proj_b, logit_scale, out_w, out_b, **kw
):
    global _CACHED_NC
    query = np.asarray(query, dtype=np.float32)
    key = np.asarray(key, dtype=np.float32)
    value = np.asarray(value, dtype=np.float32)
    in_proj_w = np.asarray(in_proj_w, dtype=np.float32)
    in_proj_b = np.asarray(in_proj_b, dtype=np.float32)
    logit_scale = np.asarray(logit_scale, dtype=np.float32)
    out_w = np.asarray(out_w, dtype=np.float32)
    out_b = np.asarray(out_b, dtype=np.float32)

    ls = np.exp(np.minimum(logit_scale.reshape(H), LOGIT_SCALE_MAX))  # (16,)

    # selector constants: sel8[t, p, e] = 1 where e == head-slot of partition p
    sel8 = np.zeros((NT, 128, 8), dtype=np.float32)
    for t in range(NT):
        for p in range(128):
            sel8[t, p, 2 * t + p // 64] = 1.0

    sel2T_h = np.zeros((2, 128), dtype=np.float32)
    sel2T_h[0, 0:64] = 1.0
    sel2T_h[1, 64:128] = 1.0

    in_maps = []
    for c in range(8):
        b, g = c // 2, c % 2
        heads = slice(g * 8, (g + 1) * 8)
        dims = slice(g * G, (g + 1) * G)
        ls_c = ls[heads]  # (8,)
        qt = np.ascontiguousarray(query[:, b, :].T)
        kt = np.ascontiguousarray(key[:, b, :].T)
        vt = np.ascontiguousarray(value[:, b, :].T)
        wq = np.ascontiguousarray(in_proj_w[0 * C :, :][dims, :].T)
        wk = np.ascontiguousarray(in_proj_w[1 * C :, :][dims, :].T)
        wv = np.ascontiguousarray(in_proj_w[2 * C :, :][dims, :].T)
        wo = np.ascontiguousarray(out_w[:, dims].T)
        bq = np.ascontiguousarray(in_proj_b[0 * C :][dims].reshape(NT, 128).T)
        bk = np.ascontiguousarray(in_proj_b[1 * C :][dims].reshape(NT, 128).T)
        bv = in_proj_b[2 * C :][dims].reshape(1, G)
        lsinv2 = (1.0 / ls_c**2).reshape(8, 1)
        # per-(partition, head) exp bias: -ls_h, constant down partitions
        lsbias = np.repeat(-ls_c.reshape(1, 8), 128, axis=0)
        in_maps.append(
            {
                "qt": qt.copy(),
                "kt": kt.copy(),
                "vt": vt.copy(),
                "wq": wq.copy(),
                "wk": wk.copy(),
                "wv": wv.copy(),
                "wo": wo.copy(),
                "bq": bq.copy(),
                "bk": bk.copy(),
                "bv": np.ascontiguousarray(bv),
                "sel8": sel8,
                "ones1": np.ones((1, 128), dtype=np.float32),
                "sel2T": sel2T_h,
                "lsinv2": np.ascontiguousarray(lsinv2, dtype=np.float32),
                "lsbias": np.ascontiguousarray(lsbias, dtype=np.float32),
            }
        )

    global _LAST_IN_MAPS
    _LAST_IN_MAPS = in_maps
    if _CACHED_NC is None:
        _CACHED_NC = build_nc()
    res = run_bass_kernel_spmd(_CACHED_NC, in_maps, core_ids=list(range(8)))

    out = np.zeros((N, B, C), dtype=np.float32)
    for c in range(8):
        b = c // 2
        out[:, b, :] += res.results[c]["out"]
    out += out_b.reshape(1, 1, C)
    return out



# revision 4
# speedup vs baseline: 1.3998x; 1.3998x over previous
# Trainium2 Bass kernel for nn_CustomAttention (cosine-sim multi-head attention).
#
# Sharding over 8 cores: core c handles batch b = c//2 and head group
# g = c%2 (8 of 16 heads, 512 feature dims).  Each core computes its heads'
# q/k/v projections (Megatron column-parallel), cosine-sim attention, and a
# partial output projection (row-parallel over its 512 dims).  The host sums
# the two partial outputs per batch and adds out_b.
#
# All heavy matmuls run in fp16 (1 cycle/row on the PE at any output width,
# unlike f32r which needs >=256 moving columns), with fp32 PSUM accumulation.
# Layout highlights:
#   qT/kT: (dims=512, seq=1024) as 4 tiles of (128, 1024); head dims on
#          partitions so the scores matmul contracts head_dim on partitions.
#   scores are computed transposed, sT[k, q]; the per-key 1/||k||*ls factor
#   and the -ls bias fold into the exp() activation as per-partition APs.
#   1/||q|| and 1/||k|| come from exp(-0.5*ln(ssq)) so the whole kernel uses
#   a single activation table (ln+exp) -- no Sqrt table reloads.
#   p@v runs transposed: out[x: q, d] = eT[k, q-block]^T @ v[k, d|1]; the
#   appended ones-column of v gives the softmax denominator, which is applied
#   as a per-partition (per-query) scalar during the PSUM->SBUF copy.
#   x tiles are then PE-transposed back to (dims, seq) for the out-proj.
#   out-proj is split into two half-contractions (t0+t1, t2+t3) so most of it
#   overlaps the attention stream; the halves are summed in SBUF.
#
# The attention main loop software-pipelines scores/exp (ACT-paced) against
# p@v, the v projection, x transposes and the first out-proj half, which are
# woven into the PE instruction stream as fillers (engine queues drain
# strictly in issue order, so issue order == execution order).

import math
import sys
from collections import deque

import numpy as np

sys.path.insert(0, "/opt/trn_rl_repo")

import concourse.bass as bass
import concourse.tile as tile
from concourse import bacc, mybir
from concourse.bass_utils import run_bass_kernel_spmd

N = 1024  # sequence length
B = 4  # batch
C = 1024  # channels
H = 16  # total heads
HD = 64  # head dim
G = 512  # dims per core (8 heads)
NT = 4  # (128, N) tiles of qT/kT per core
CC = 8  # contraction chunks of 128 over C
ST = 8  # seq tiles of 128
QC = 2  # seq chunks of 512
LOGIT_SCALE_MAX = math.log(1.0 / 0.01)

F32 = mybir.dt.float32
F32R = mybir.dt.float32r
F16 = mybir.dt.float16
AF = mybir.ActivationFunctionType

_CACHED_NC = None
_LAST_IN_MAPS = None


def build_nc():
    nc = bacc.Bacc("TRN2", target_bir_lowering=False)

    qt_d = nc.declare_dram_parameter("qt", [C, N], F16, isOutput=False)
    kt_d = nc.declare_dram_parameter("kt", [C, N], F16, isOutput=False)
    vt_d = nc.declare_dram_parameter("vt", [C, N], F16, isOutput=False)
    wq_d = nc.declare_dram_parameter("wq", [C, G], F16, isOutput=False)
    wk_d = nc.declare_dram_parameter("wk", [C, G], F16, isOutput=False)
    wv_d = nc.declare_dram_parameter("wv", [C, G], F16, isOutput=False)
    wo_d = nc.declare_dram_parameter("wo", [G, C], F16, isOutput=False)
    bq_d = nc.declare_dram_parameter("bq", [128, NT], F32, isOutput=False)
    bk_d = nc.declare_dram_parameter("bk", [128, NT], F32, isOutput=False)
    bv_d = nc.declare_dram_parameter("bv", [1, G], F16, isOutput=False)
    sel8_d = nc.declare_dram_parameter("sel8", [NT, 128, 8], F16, isOutput=False)
    sel8T_d = nc.declare_dram_parameter("sel8T", [8, NT, 128], F32R, isOutput=False)
    lnls_d = nc.declare_dram_parameter("lnls", [8, 1], F32, isOutput=False)
    lsbias_d = nc.declare_dram_parameter("lsbias", [128, 8], F32, isOutput=False)
    ident_d = nc.declare_dram_parameter("ident", [128, 128], F32R, isOutput=False)
    ones1_d = nc.declare_dram_parameter("ones1", [1, 128], F16, isOutput=False)
    out_d = nc.declare_dram_parameter("out", [N, C], F32, isOutput=True)

    qt_r = qt_d[:].rearrange("(cc p) n -> cc p n", p=128)
    kt_r = kt_d[:].rearrange("(cc p) n -> cc p n", p=128)
    vt_r = vt_d[:].rearrange("(cc p) n -> cc p n", p=128)
    # weight halves: [2, 128, 4, G]
    wq_r = wq_d[:].rearrange("(g cc p) o -> g p cc o", g=2, p=128)
    wk_r = wk_d[:].rearrange("(g cc p) o -> g p cc o", g=2, p=128)
    wv_r = wv_d[:].rearrange("(g cc p) o -> g p cc o", g=2, p=128)

    with tile.TileContext(nc) as tc:
        with (
            tc.tile_pool(name="consts", bufs=1) as consts,
            tc.tile_pool(name="wo_p", bufs=1) as wo_p,
            tc.tile_pool(name="w_p", bufs=3) as w_p,
            tc.tile_pool(name="acts", bufs=18) as acts,
            tc.tile_pool(name="big", bufs=1) as big,
            tc.tile_pool(name="sq_p", bufs=2) as sq_p,
            tc.tile_pool(name="stats", bufs=1) as stats,
            tc.tile_pool(name="lssq_p", bufs=2) as lssq_p,
            tc.tile_pool(name="eT_p", bufs=18) as eT_p,
            tc.tile_pool(name="x_p", bufs=2) as x_p,
            tc.tile_pool(name="den_p", bufs=2) as den_p,
            tc.tile_pool(name="oA_p", bufs=1) as oA_p,
        ):
            # ---- persistent tiles ----
            qT = [big.tile([128, N], F16, tag=f"qT{t}", name=f"qT{t}") for t in range(NT)]
            kT = [big.tile([128, N], F16, tag=f"kT{t}", name=f"kT{t}") for t in range(NT)]
            v_sb = [big.tile([128, 8, HD + 1], F16, tag=f"v{s}", name=f"v{s}") for s in range(ST)]
            xt = [big.tile([128, N], F16, tag=f"xt{t}", name=f"xt{t}") for t in range(NT)]
            rskT = stats.tile([128, ST, 8], F32)
            rsq = stats.tile([8, N], F32R)
            rsk = stats.tile([8, N], F32R)
            oA = oA_p.tile([128, ST, 2, G], F32)

            # ones column of v (softmax denominator); disjoint from the
            # projection writes so it can be set up front.
            for s in range(ST):
                nc.gpsimd.memset(v_sb[s][:, :, HD], 1.0)

            # ---- DMA stream (single SP queue; issue order = transfer order) ----
            def stream_acts(dram_r, nm):
                ch = []
                for cc in range(CC):
                    a = acts.tile([128, N], F16, tag="act", name=f"{nm}{cc}")
                    nc.sync.dma_start(out=a[:], in_=dram_r[cc])
                    ch.append(a)
                return ch

            # q stream: first act chunks interleaved with the weight halves
            wq_sb = w_p.tile([128, CC, G], F16, tag="w", name="wq")
            qch = []
            a0 = acts.tile([128, N], F16, tag="act", name="qt0")
            nc.sync.dma_start(out=a0[:], in_=qt_r[0])
            qch.append(a0)
            nc.sync.dma_start(out=wq_sb[:, 0:4, :], in_=wq_r[0])
            for cc in range(1, 4):
                a = acts.tile([128, N], F16, tag="act", name=f"qt{cc}")
                nc.sync.dma_start(out=a[:], in_=qt_r[cc])
                qch.append(a)
            nc.sync.dma_start(out=wq_sb[:, 4:8, :], in_=wq_r[1])
            for cc in range(4, 8):
                a = acts.tile([128, N], F16, tag="act", name=f"qt{cc}")
                nc.sync.dma_start(out=a[:], in_=qt_r[cc])
                qch.append(a)

            # consts needed during the q projection
            sel8 = consts.tile([128, NT, 8], F16)
            nc.sync.dma_start(out=sel8[:], in_=sel8_d[:].rearrange("t p e -> p t e"))
            bq_sb = consts.tile([128, NT], F32)
            nc.sync.dma_start(out=bq_sb[:], in_=bq_d[:])

            wk_sb = w_p.tile([128, CC, G], F16, tag="w", name="wk")
            nc.sync.dma_start(out=wk_sb[:, 0:4, :], in_=wk_r[0])
            nc.sync.dma_start(out=wk_sb[:, 4:8, :], in_=wk_r[1])
            kch = stream_acts(kt_r, "kt")

            sel8T = consts.tile([8, NT, 128], F32R)
            nc.sync.dma_start(out=sel8T[:], in_=sel8T_d[:])
            bk_sb = consts.tile([128, NT], F32)
            nc.sync.dma_start(out=bk_sb[:], in_=bk_d[:])
            lnls = consts.tile([8, 1], F32)
            nc.sync.dma_start(out=lnls[:], in_=lnls_d[:])
            ident = consts.tile([128, 128], F32R)
            nc.sync.dma_start(out=ident[:], in_=ident_d[:])

            wv_sb = w_p.tile([128, CC, G], F16, tag="w", name="wv")
            nc.sync.dma_start(out=wv_sb[:, 0:4, :], in_=wv_r[0])
            nc.sync.dma_start(out=wv_sb[:, 4:8, :], in_=wv_r[1])
            vch = stream_acts(vt_r, "vt")

            lsbias = consts.tile([128, 8], F32)
            nc.sync.dma_start(out=lsbias[:], in_=lsbias_d[:])
            ones1 = consts.tile([1, 128], F16)
            nc.sync.dma_start(out=ones1[:], in_=ones1_d[:])
            bv_sb = consts.tile([1, G], F16)
            nc.sync.dma_start(out=bv_sb[:], in_=bv_d[:])
            wo_sb = wo_p.tile([128, NT, C], F16)
            nc.sync.dma_start(
                out=wo_sb[:], in_=wo_d[:].rearrange("(t p) c -> p t c", p=128)
            )

            # ---- phase 1: q/k projections + norms ----
            deferred = deque()

            def flush(n=99):
                for _ in range(min(n, len(deferred))):
                    deferred.popleft()()

            with (
                tc.tile_pool(name="pp", bufs=4, space="PSUM") as pp,
                tc.tile_pool(name="pssq", bufs=2, space="PSUM") as pssq_p,
                tc.tile_pool(name="pbc", bufs=2, space="PSUM") as pbc,
            ):
                def proj_wave(w_sb, ch, dst, b_sb, ssq_half, copy_eng, qc, tp):
                    pst = [pp.tile([128, G], F32, tag="proj", name=f"proj{qc}{tp}{i}") for i in range(2)]
                    for cc in range(CC):
                        for ti in range(2):
                            t = 2 * tp + ti
                            nc.tensor.matmul(
                                pst[ti][:],
                                w_sb[:, cc, t * 128 : (t + 1) * 128],
                                ch[cc][:, qc * 512 : (qc + 1) * 512],
                                start=(cc == 0),
                                stop=(cc == CC - 1),
                            )
                    for ti in range(2):
                        t = 2 * tp + ti
                        copy_eng.tensor_scalar_add(
                            out=dst[t][:, qc * 512 : (qc + 1) * 512],
                            in0=pst[ti][:],
                            scalar1=b_sb[:, t : t + 1],
                        )
                        sq = sq_p.tile([128, G], F16, tag="sq", name=f"sq{t}{qc}")
                        nc.vector.tensor_mul(
                            out=sq[:],
                            in0=dst[t][:, qc * 512 : (qc + 1) * 512],
                            in1=dst[t][:, qc * 512 : (qc + 1) * 512],
                        )

                        def ssq_mm(t=t, sq=sq):
                            nc.tensor.matmul(
                                ssq_half[:],
                                sel8[:, t, :],
                                sq[:],
                                start=(t == 0),
                                stop=(t == NT - 1),
                            )

                        deferred.append(ssq_mm)

                # q projection; norms per qc half: rsq = exp(-0.5*ln(ssq))
                def q_norm_thunk(ssq_half, qc):
                    def run():
                        lssq = lssq_p.tile([8, G], F32, tag="lssq", name=f"lssq_q{qc}")
                        nc.scalar.activation(out=lssq[:], in_=ssq_half[:], func=AF.Ln)
                        nc.scalar.activation(
                            out=rsq[:, qc * 512 : (qc + 1) * 512],
                            in_=lssq[:], func=AF.Exp, scale=-0.5,
                        )

                    return run

                def bc_thunk(t, qc):
                    def run():
                        pb = pbc.tile([128, G], F32, tag="bc", name=f"bc{t}{qc}")
                        nc.tensor.matmul(
                            pb[:],
                            sel8T[:, t, :],
                            rsq[:, qc * 512 : (qc + 1) * 512],
                            start=True,
                            stop=True,
                        )
                        nc.vector.tensor_mul(
                            out=qT[t][:, qc * 512 : (qc + 1) * 512],
                            in0=qT[t][:, qc * 512 : (qc + 1) * 512],
                            in1=pb[:],
                        )

                    return run

                for qc in range(QC):
                    ssq_half = pssq_p.tile([8, G], F32, tag="ssq", name=f"ssq_q{qc}")
                    for tp in range(2):
                        proj_wave(wq_sb, qch, qT, bq_sb, ssq_half, nc.vector, qc, tp)
                        flush(2)
                    flush()
                    deferred.append(q_norm_thunk(ssq_half, qc))
                    for t in range(NT):
                        deferred.append(bc_thunk(t, qc))

                # k projection (psum->sbuf copies on Pool to spread DVE load);
                # norms + rskT transposes per qc half so the first half's chain
                # overlaps the second half's waves.
                def k_norm_thunk(ssq_half, qc):
                    def run():
                        lssq = lssq_p.tile([8, G], F32, tag="lssq", name=f"lssq_k{qc}")
                        nc.scalar.activation(out=lssq[:], in_=ssq_half[:], func=AF.Ln)
                        nc.scalar.activation(
                            out=rsk[:, qc * 512 : (qc + 1) * 512],
                            in_=lssq[:], func=AF.Exp, scale=-0.5,
                            bias=lnls[:, 0:1],
                        )

                    return run

                def rskT_thunk(s):
                    def run():
                        pt = pbc.tile([128, G], F32, tag="bc", name=f"rskt{s}")
                        nc.tensor.transpose(
                            pt[:].bitcast(F32R)[:, 0:8],
                            rsk[:, s * 128 : (s + 1) * 128],
                            ident[0:8, 0:8],
                        )
                        nc.gpsimd.tensor_copy(out=rskT[:, s, :], in_=pt[:, 0:8])

                    return run

                for qc in range(QC):
                    ssq_half = pssq_p.tile([8, G], F32, tag="ssq", name=f"ssq_k{qc}")
                    for tp in range(2):
                        proj_wave(wk_sb, kch, kT, bk_sb, ssq_half, nc.gpsimd, qc, tp)
                        flush(2)
                    flush()
                    deferred.append(k_norm_thunk(ssq_half, qc))
                    for s in range(4 * qc, 4 * qc + 4):
                        deferred.append(rskT_thunk(s))
                flush()

            # ---- phase 2: attention + out-projection ----
            with (
                tc.tile_pool(name="psT", bufs=2, space="PSUM") as psT_p,
                tc.tile_pool(name="pv", bufs=1, space="PSUM") as pv_p,
                tc.tile_pool(name="po", bufs=2, space="PSUM") as po_p,
            ):
                ets = {}
                cur_pv = {}
                cur_x = [None] * 8
                vp_issued = [False] * ST
                fill_hi = deque()
                fill_lo = deque()

                def sc(h, s):
                    t, j = divmod(h, 2)
                    st = psT_p.tile([128, N], F32, tag="sT", name=f"sT{h}_{s}")
                    for qc in range(QC):
                        nc.tensor.matmul(
                            st[:, qc * 512 : (qc + 1) * 512],
                            kT[t][j * 64 : (j + 1) * 64, s * 128 : (s + 1) * 128],
                            qT[t][j * 64 : (j + 1) * 64, qc * 512 : (qc + 1) * 512],
                            start=True,
                            stop=True,
                        )
                    e = eT_p.tile([128, N], F16, tag="eT", name=f"eT{h}_{s}")
                    nc.scalar.activation(
                        out=e[:], in_=st[:], func=AF.Exp,
                        bias=lsbias[:, h : h + 1],
                        scale=rskT[:, s, h : h + 1],
                    )
                    ets[(h, s)] = e

                def vp_wave(s):
                    ps = po_p.tile([128, G], F32, tag="po", name=f"vp{s}")
                    for cc in range(CC):
                        nc.tensor.matmul(
                            ps[:],
                            vch[cc][:, s * 128 : (s + 1) * 128],
                            wv_sb[:, cc, :],
                            start=(cc == 0),
                            stop=False,
                        )
                    nc.tensor.matmul(ps[:], ones1[:], bv_sb[:], start=False, stop=True)
                    nc.gpsimd.tensor_copy(
                        out=v_sb[s][:, :, 0:HD],
                        in_=ps[:].rearrange("p (h d) -> p h d", h=8),
                    )
                    vp_issued[s] = True

                def pv_op(h, s):
                    pvA, pvB = cur_pv[h]
                    e = ets.pop((h, s))
                    for qb in range(8):
                        grp = pvA if qb < 4 else pvB
                        nc.tensor.matmul(
                            grp[:, qb % 4, :],
                            e[:, qb * 128 : (qb + 1) * 128],
                            v_sb[s][:, h, :],
                            start=(s == 0),
                            stop=(s == ST - 1),
                        )

                def transp(t, qb, xx):
                    pt = po_p.tile([128, G], F32, tag="po", name=f"tp{t}_{qb}")
                    nc.tensor.transpose(
                        pt[:].bitcast(F32R)[:, 0:128], xx[:], ident[:]
                    )
                    eng = nc.vector if qb % 2 else nc.gpsimd
                    eng.tensor_copy(
                        out=xt[t][:, qb * 128 : (qb + 1) * 128], in_=pt[:, 0:128]
                    )

                def out_pass(s_, coc, ts, first):
                    ps = po_p.tile([128, G], F32, tag="po", name=f"op{s_}_{coc}")
                    for i, t_ in enumerate(ts):
                        nc.tensor.matmul(
                            ps[:],
                            xt[t_][:, s_ * 128 : (s_ + 1) * 128],
                            wo_sb[:, t_, coc * 512 : (coc + 1) * 512],
                            start=(i == 0),
                            stop=(i == len(ts) - 1),
                        )
                    eng = nc.vector if (s_ + coc) % 2 == 0 else nc.gpsimd
                    if first:
                        eng.tensor_copy(out=oA[:, s_, coc, :], in_=ps[:])
                    else:
                        eng.tensor_add(
                            out=oA[:, s_, coc, :], in0=ps[:], in1=oA[:, s_, coc, :]
                        )
                        nc.sync.dma_start(
                            out=out_d[:][
                                s_ * 128 : (s_ + 1) * 128, coc * 512 : (coc + 1) * 512
                            ],
                            in_=oA[:, s_, coc, :],
                        )

                def head_end(h):
                    t, j = divmod(h, 2)
                    pvA, pvB = cur_pv.pop(h)
                    den = den_p.tile([128, 8], F32, tag="den", name=f"den{h}")
                    nc.vector.tensor_copy(out=den[:, 0:4], in_=pvA[:, :, HD])
                    nc.vector.tensor_copy(out=den[:, 4:8], in_=pvB[:, :, HD])
                    rden = den_p.tile([128, 8], F32, tag="rden", name=f"rden{h}")
                    nc.vector.reciprocal(out=rden[:], in_=den[:])
                    for qb in range(8):
                        if j == 0:
                            cur_x[qb] = x_p.tile(
                                [128, 128], F32R, tag=f"x{qb}", name=f"x{qb}_{t}"
                            )
                        grp = pvA if qb < 4 else pvB
                        eng = nc.vector if qb < 4 else nc.gpsimd
                        eng.tensor_scalar_mul(
                            out=cur_x[qb][:, j * 64 : (j + 1) * 64],
                            in0=grp[:, qb % 4, 0:HD],
                            scalar1=rden[:, qb : qb + 1],
                        )
                    if j == 1:
                        for qb in range(8):
                            fill_hi.append(
                                lambda t=t, qb=qb, xx=cur_x[qb]: transp(t, qb, xx)
                            )
                        if t == 1:
                            for s_ in range(ST):
                                for coc in range(2):
                                    fill_lo.append(
                                        lambda s_=s_, coc=coc: out_pass(
                                            s_, coc, (0, 1), True
                                        )
                                    )

                # v-proj waves are scheduled at fixed steps across h0..h2
                vp_sched = {3 * i: i for i in range(ST)}
                pv_ptr = 0

                def pv_ready(ptr, g):
                    h_, s_ = divmod(ptr, 8)
                    if h_ == 0 and not vp_issued[s_]:
                        return False
                    lag = 4 if s_ == 0 else 2
                    return g >= 8 * h_ + s_ + lag if h_ > 0 else g >= s_ + lag

                def drain_pv(g, budget=3):
                    nonlocal pv_ptr
                    while budget > 0 and pv_ptr < 64 and pv_ready(pv_ptr, g):
                        h_, s_ = divmod(pv_ptr, 8)
                        if s_ == 0:
                            cur_pv[h_] = (
                                pv_p.tile([128, 4, HD + 1], F32, tag="pvA", name=f"pvA{h_}"),
                                pv_p.tile([128, 4, HD + 1], F32, tag="pvB", name=f"pvB{h_}"),
                            )
                        pv_op(h_, s_)
                        pv_ptr += 1
                        if s_ == ST - 1:
                            head_end(h_)
                        budget -= 1

                for g in range(64):
                    h, s = divmod(g, 8)
                    sc(h, s)
                    if g in vp_sched:
                        vp_wave(vp_sched[g])
                    elif fill_hi:
                        fill_hi.popleft()()
                    elif fill_lo:
                        fill_lo.popleft()()
                    drain_pv(g)

                # drain: remaining pv ops, last pair's transposes, out-proj B
                while pv_ptr < 64:
                    drain_pv(99, budget=8)
                while fill_hi:
                    fill_hi.popleft()()
                while fill_lo:
                    fill_lo.popleft()()
                for s_ in range(ST):
                    stile = psT_p.tile([128, N], F32, tag="sT", name=f"ob{s_}")
                    for coc in range(2):
                        for i, t_ in enumerate((2, 3)):
                            nc.tensor.matmul(
                                stile[:, coc * 512 : (coc + 1) * 512],
                                xt[t_][:, s_ * 128 : (s_ + 1) * 128],
                                wo_sb[:, t_, coc * 512 : (coc + 1) * 512],
                                start=(i == 0),
                                stop=(i == 1),
                            )
                    for coc in range(2):
                        eng = nc.vector if coc == 0 else nc.gpsimd
                        eng.tensor_add(
                            out=oA[:, s_, coc, :],
                            in0=stile[:, coc * 512 : (coc + 1) * 512],
                            in1=oA[:, s_, coc, :],
                        )
                        nc.sync.dma_start(
                            out=out_d[:][
                                s_ * 128 : (s_ + 1) * 128, coc * 512 : (coc + 1) * 512
                            ],
                            in_=oA[:, s_, coc, :],
                        )

    nc.compile()
    return nc


def kernel(
    query, key, value, in_proj_w, in_proj_b, logit_scale, out_w, out_b, **kw
):
    global _CACHED_NC, _LAST_IN_MAPS
    query = np.asarray(query, dtype=np.float32)
    key = np.asarray(key, dtype=np.float32)
    value = np.asarray(value, dtype=np.float32)
    in_proj_w = np.asarray(in_proj_w, dtype=np.float32)
    in_proj_b = np.asarray(in_proj_b, dtype=np.float32)
    logit_scale = np.asarray(logit_scale, dtype=np.float32)
    out_w = np.asarray(out_w, dtype=np.float32)
    out_b = np.asarray(out_b, dtype=np.float32)

    ls = np.exp(np.minimum(logit_scale.reshape(H), LOGIT_SCALE_MAX))  # (16,)

    # selector constants
    sel8 = np.zeros((NT, 128, 8), dtype=np.float16)
    sel8T = np.zeros((8, NT, 128), dtype=np.float32)
    for t in range(NT):
        for p in range(128):
            h = 2 * t + p // 64
            sel8[t, p, h] = 1.0
            sel8T[h, t, p] = 1.0
    ident = np.eye(128, dtype=np.float32)

    in_maps = []
    for c in range(8):
        b, g = c // 2, c % 2
        dims = slice(g * G, (g + 1) * G)
        ls_c = ls[g * 8 : (g + 1) * 8]  # (8,)
        qt = np.ascontiguousarray(query[:, b, :].T, dtype=np.float16)
        kt = np.ascontiguousarray(key[:, b, :].T, dtype=np.float16)
        vt = np.ascontiguousarray(value[:, b, :].T, dtype=np.float16)
        wq = np.ascontiguousarray(in_proj_w[0 * C :, :][dims, :].T, dtype=np.float16)
        wk = np.ascontiguousarray(in_proj_w[1 * C :, :][dims, :].T, dtype=np.float16)
        wv = np.ascontiguousarray(in_proj_w[2 * C :, :][dims, :].T, dtype=np.float16)
        wo = np.ascontiguousarray(out_w[:, dims].T, dtype=np.float16)
        bq = np.ascontiguousarray(in_proj_b[0 * C :][dims].reshape(NT, 128).T)
        bk = np.ascontiguousarray(in_proj_b[1 * C :][dims].reshape(NT, 128).T)
        bv = np.ascontiguousarray(
            in_proj_b[2 * C :][dims].reshape(1, G), dtype=np.float16
        )
        lnls = np.log(ls_c).reshape(8, 1).astype(np.float32)
        lsbias = np.repeat(-ls_c.reshape(1, 8), 128, axis=0).astype(np.float32)
        in_maps.append(
            {
                "qt": qt,
                "kt": kt,
                "vt": vt,
                "wq": wq,
                "wk": wk,
                "wv": wv,
                "wo": wo,
                "bq": bq.astype(np.float32),
                "bk": bk.astype(np.float32),
                "bv": bv,
                "sel8": sel8,
                "sel8T": sel8T,
                "lnls": lnls,
                "lsbias": lsbias,
                "ident": ident,
                "ones1": np.ones((1, 128), dtype=np.float16),
            }
        )

    _LAST_IN_MAPS = in_maps
    if _CACHED_NC is None:
        _CACHED_NC = build_nc()
    res = run_bass_kernel_spmd(_CACHED_NC, in_maps, core_ids=list(range(8)))

    out = np.zeros((N, B, C), dtype=np.float32)
    for c in range(8):
        b = c // 2
        out[:, b, :] += res.results[c]["out"]
    out += out_b.reshape(1, 1, C)
    return out


# revision 5
# speedup vs baseline: 1.4149x; 1.0108x over previous
# Trainium2 Bass kernel for nn_CustomAttention (cosine-sim multi-head attention).
#
# Sharding over 8 cores: core c handles batch b = c//2 and head group
# g = c%2 (8 of 16 heads, 512 feature dims).  Each core computes its heads'
# q/k/v projections (Megatron column-parallel), cosine-sim attention, and a
# partial output projection (row-parallel over its 512 dims).  The host sums
# the two partial outputs per batch and adds out_b.
#
# All heavy matmuls run in fp16 (1 cycle/row on the PE at any output width,
# unlike f32r which needs >=256 moving columns), with fp32 PSUM accumulation.
# Layout highlights:
#   qT/kT: (dims=512, seq=1024) as 4 tiles of (128, 1024); head dims on
#          partitions so the scores matmul contracts head_dim on partitions.
#   scores are computed transposed, sT[k, q]; the per-key 1/||k||*ls factor
#   and the -ls bias fold into the exp() activation as per-partition APs.
#   1/||q|| and 1/||k|| come from exp(-0.5*ln(ssq)) so the whole kernel uses
#   a single activation table (ln+exp) -- no Sqrt table reloads.
#   p@v runs transposed: out[x: q, d] = eT[k, q-block]^T @ v[k, d|1]; the
#   appended ones-column of v gives the softmax denominator, which is applied
#   as a per-partition (per-query) scalar during the PSUM->SBUF copy.
#   x tiles are then PE-transposed back to (dims, seq) for the out-proj.
#   out-proj is split into two half-contractions (t0+t1, t2+t3) so most of it
#   overlaps the attention stream; the halves are summed in SBUF.
#
# The attention main loop software-pipelines scores/exp (ACT-paced) against
# p@v, the v projection, x transposes and the first out-proj half, which are
# woven into the PE instruction stream as fillers (engine queues drain
# strictly in issue order, so issue order == execution order).

import math
import sys
from collections import deque

import numpy as np

sys.path.insert(0, "/opt/trn_rl_repo")

import concourse.bass as bass
import concourse.tile as tile
from concourse import bacc, mybir
from concourse.bass_utils import run_bass_kernel_spmd

N = 1024  # sequence length
B = 4  # batch
C = 1024  # channels
H = 16  # total heads
HD = 64  # head dim
G = 512  # dims per core (8 heads)
NT = 4  # (128, N) tiles of qT/kT per core
CC = 8  # contraction chunks of 128 over C
ST = 8  # seq tiles of 128
QC = 2  # seq chunks of 512
LOGIT_SCALE_MAX = math.log(1.0 / 0.01)

F32 = mybir.dt.float32
F32R = mybir.dt.float32r
F16 = mybir.dt.float16
AF = mybir.ActivationFunctionType

_CACHED_NC = None
_LAST_IN_MAPS = None


def build_nc():
    nc = bacc.Bacc("TRN2", target_bir_lowering=False)

    qt_d = nc.declare_dram_parameter("qt", [C, N], F16, isOutput=False)
    kt_d = nc.declare_dram_parameter("kt", [C, N], F16, isOutput=False)
    vt_d = nc.declare_dram_parameter("vt", [C, N], F16, isOutput=False)
    wq_d = nc.declare_dram_parameter("wq", [C, G], F16, isOutput=False)
    wk_d = nc.declare_dram_parameter("wk", [C, G], F16, isOutput=False)
    wv_d = nc.declare_dram_parameter("wv", [C, G], F16, isOutput=False)
    wo_d = nc.declare_dram_parameter("wo", [G, C], F16, isOutput=False)
    bq_d = nc.declare_dram_parameter("bq", [128, NT], F32, isOutput=False)
    bk_d = nc.declare_dram_parameter("bk", [128, NT], F32, isOutput=False)
    bv_d = nc.declare_dram_parameter("bv", [1, G], F16, isOutput=False)
    sel8_d = nc.declare_dram_parameter("sel8", [NT, 128, 8], F16, isOutput=False)
    sel8T_d = nc.declare_dram_parameter("sel8T", [8, NT, 128], F32R, isOutput=False)
    lnls_d = nc.declare_dram_parameter("lnls", [8, 1], F32, isOutput=False)
    lsbias_d = nc.declare_dram_parameter("lsbias", [128, 8], F32, isOutput=False)
    ident_d = nc.declare_dram_parameter("ident", [128, 128], F32R, isOutput=False)
    ones1_d = nc.declare_dram_parameter("ones1", [1, 128], F16, isOutput=False)
    out_d = nc.declare_dram_parameter("out", [N, C], F32, isOutput=True)

    qt_r = qt_d[:].rearrange("(cc p) n -> cc p n", p=128)
    kt_r = kt_d[:].rearrange("(cc p) n -> cc p n", p=128)
    vt_r = vt_d[:].rearrange("(cc p) n -> cc p n", p=128)
    # weight halves: [2, 128, 4, G]
    wq_r = wq_d[:].rearrange("(g cc p) o -> g p cc o", g=2, p=128)
    wk_r = wk_d[:].rearrange("(g cc p) o -> g p cc o", g=2, p=128)
    wv_r = wv_d[:].rearrange("(g cc p) o -> g p cc o", g=2, p=128)

    # pre-load the ln+exp activation table once; every activation in this
    # kernel (Ln, Exp) is servable from it, so the auto-inserted loads
    # (which thrash between exp-only and ln-only tables) are avoided.
    from concourse.hw_specs import get_activation_tables

    table_names = list(get_activation_tables(nc.m.arch).keys())
    lnexp_id = table_names.index("natural_log_exp_and_others")

    with tile.TileContext(nc) as tc:
        nc.scalar.add_instruction(
            mybir.InstLoadActFuncSet(
                name=nc.get_next_instruction_name(), ins=[], outs=[],
                act_func_set_id=lnexp_id,
            )
        )
        with (
            tc.tile_pool(name="consts", bufs=1) as consts,
            tc.tile_pool(name="wo_p", bufs=1) as wo_p,
            tc.tile_pool(name="w_p", bufs=3) as w_p,
            tc.tile_pool(name="acts", bufs=18) as acts,
            tc.tile_pool(name="big", bufs=1) as big,
            tc.tile_pool(name="sq_p", bufs=2) as sq_p,
            tc.tile_pool(name="stats", bufs=1) as stats,
            tc.tile_pool(name="lssq_p", bufs=2) as lssq_p,
            tc.tile_pool(name="eT_p", bufs=18) as eT_p,
            tc.tile_pool(name="x_p", bufs=2) as x_p,
            tc.tile_pool(name="den_p", bufs=2) as den_p,
            tc.tile_pool(name="oA_p", bufs=1) as oA_p,
        ):
            # ---- persistent tiles ----
            qT = [big.tile([128, N], F16, tag=f"qT{t}", name=f"qT{t}") for t in range(NT)]
            kT = [big.tile([128, N], F16, tag=f"kT{t}", name=f"kT{t}") for t in range(NT)]
            v_sb = [big.tile([128, 8, HD + 1], F16, tag=f"v{s}", name=f"v{s}") for s in range(ST)]
            xt = [big.tile([128, N], F16, tag=f"xt{t}", name=f"xt{t}") for t in range(NT)]
            rskT = stats.tile([128, ST, 8], F32)
            rsq = stats.tile([8, N], F32R)
            rsk = stats.tile([8, N], F32R)
            oA = oA_p.tile([128, ST, 2, G], F32)

            # ones column of v (softmax denominator); disjoint from the
            # projection writes so it can be set up front.
            for s in range(ST):
                nc.gpsimd.memset(v_sb[s][:, :, HD], 1.0)

            # ---- DMA stream (single SP queue; issue order = transfer order) ----
            def stream_acts(dram_r, nm):
                ch = []
                for cc in range(CC):
                    a = acts.tile([128, N], F16, tag="act", name=f"{nm}{cc}")
                    nc.sync.dma_start(out=a[:], in_=dram_r[cc])
                    ch.append(a)
                return ch

            # q stream: first act chunks interleaved with the weight halves
            wq_sb = w_p.tile([128, CC, G], F16, tag="w", name="wq")
            qch = []
            a0 = acts.tile([128, N], F16, tag="act", name="qt0")
            nc.sync.dma_start(out=a0[:], in_=qt_r[0])
            qch.append(a0)
            nc.sync.dma_start(out=wq_sb[:, 0:4, :], in_=wq_r[0])
            for cc in range(1, 4):
                a = acts.tile([128, N], F16, tag="act", name=f"qt{cc}")
                nc.sync.dma_start(out=a[:], in_=qt_r[cc])
                qch.append(a)
            nc.sync.dma_start(out=wq_sb[:, 4:8, :], in_=wq_r[1])
            for cc in range(4, 8):
                a = acts.tile([128, N], F16, tag="act", name=f"qt{cc}")
                nc.sync.dma_start(out=a[:], in_=qt_r[cc])
                qch.append(a)

            # consts needed during the q projection
            sel8 = consts.tile([128, NT, 8], F16)
            nc.sync.dma_start(out=sel8[:], in_=sel8_d[:].rearrange("t p e -> p t e"))
            bq_sb = consts.tile([128, NT], F32)
            nc.sync.dma_start(out=bq_sb[:], in_=bq_d[:])

            wk_sb = w_p.tile([128, CC, G], F16, tag="w", name="wk")
            nc.sync.dma_start(out=wk_sb[:, 0:4, :], in_=wk_r[0])
            nc.sync.dma_start(out=wk_sb[:, 4:8, :], in_=wk_r[1])
            kch = stream_acts(kt_r, "kt")

            sel8T = consts.tile([8, NT, 128], F32R)
            nc.sync.dma_start(out=sel8T[:], in_=sel8T_d[:])
            bk_sb = consts.tile([128, NT], F32)
            nc.sync.dma_start(out=bk_sb[:], in_=bk_d[:])
            lnls = consts.tile([8, 1], F32)
            nc.sync.dma_start(out=lnls[:], in_=lnls_d[:])
            ident = consts.tile([128, 128], F32R)
            nc.sync.dma_start(out=ident[:], in_=ident_d[:])

            wv_sb = w_p.tile([128, CC, G], F16, tag="w", name="wv")
            nc.sync.dma_start(out=wv_sb[:, 0:4, :], in_=wv_r[0])
            nc.sync.dma_start(out=wv_sb[:, 4:8, :], in_=wv_r[1])
            vch = stream_acts(vt_r, "vt")

            lsbias = consts.tile([128, 8], F32)
            nc.sync.dma_start(out=lsbias[:], in_=lsbias_d[:])
            ones1 = consts.tile([1, 128], F16)
            nc.sync.dma_start(out=ones1[:], in_=ones1_d[:])
            bv_sb = consts.tile([1, G], F16)
            nc.sync.dma_start(out=bv_sb[:], in_=bv_d[:])
            wo_sb = wo_p.tile([128, NT, C], F16)
            nc.sync.dma_start(
                out=wo_sb[:], in_=wo_d[:].rearrange("(t p) c -> p t c", p=128)
            )

            # ---- phase 1: q/k projections + norms ----
            deferred = deque()

            def flush(n=99):
                for _ in range(min(n, len(deferred))):
                    deferred.popleft()()

            with (
                tc.tile_pool(name="pp", bufs=4, space="PSUM") as pp,
                tc.tile_pool(name="pssq", bufs=2, space="PSUM") as pssq_p,
                tc.tile_pool(name="pbc", bufs=2, space="PSUM") as pbc,
            ):
                def proj_wave(w_sb, ch, dst, b_sb, ssq_half, copy_eng, qc, tp):
                    pst = [pp.tile([128, G], F32, tag="proj", name=f"proj{qc}{tp}{i}") for i in range(2)]
                    for cc in range(CC):
                        for ti in range(2):
                            t = 2 * tp + ti
                            nc.tensor.matmul(
                                pst[ti][:],
                                w_sb[:, cc, t * 128 : (t + 1) * 128],
                                ch[cc][:, qc * 512 : (qc + 1) * 512],
                                start=(cc == 0),
                                stop=(cc == CC - 1),
                            )
                    for ti in range(2):
                        t = 2 * tp + ti
                        copy_eng.tensor_scalar_add(
                            out=dst[t][:, qc * 512 : (qc + 1) * 512],
                            in0=pst[ti][:],
                            scalar1=b_sb[:, t : t + 1],
                        )
                        sq = sq_p.tile([128, G], F16, tag="sq", name=f"sq{t}{qc}")
                        nc.vector.tensor_mul(
                            out=sq[:],
                            in0=dst[t][:, qc * 512 : (qc + 1) * 512],
                            in1=dst[t][:, qc * 512 : (qc + 1) * 512],
                        )

                        def ssq_mm(t=t, sq=sq):
                            nc.tensor.matmul(
                                ssq_half[:],
                                sel8[:, t, :],
                                sq[:],
                                start=(t == 0),
                                stop=(t == NT - 1),
                            )

                        deferred.append(ssq_mm)

                # q projection; norms per qc half: rsq = exp(-0.5*ln(ssq))
                def q_norm_thunk(ssq_half, qc):
                    def run():
                        lssq = lssq_p.tile([8, G], F32, tag="lssq", name=f"lssq_q{qc}")
                        nc.scalar.activation(out=lssq[:], in_=ssq_half[:], func=AF.Ln)
                        nc.scalar.activation(
                            out=rsq[:, qc * 512 : (qc + 1) * 512],
                            in_=lssq[:], func=AF.Exp, scale=-0.5,
                        )

                    return run

                def bc_thunk(t, qc):
                    def run():
                        pb = pbc.tile([128, G], F32, tag="bc", name=f"bc{t}{qc}")
                        nc.tensor.matmul(
                            pb[:],
                            sel8T[:, t, :],
                            rsq[:, qc * 512 : (qc + 1) * 512],
                            start=True,
                            stop=True,
                        )
                        nc.vector.tensor_mul(
                            out=qT[t][:, qc * 512 : (qc + 1) * 512],
                            in0=qT[t][:, qc * 512 : (qc + 1) * 512],
                            in1=pb[:],
                        )

                    return run

                for qc in range(QC):
                    ssq_half = pssq_p.tile([8, G], F32, tag="ssq", name=f"ssq_q{qc}")
                    for tp in range(2):
                        proj_wave(wq_sb, qch, qT, bq_sb, ssq_half, nc.vector, qc, tp)
                        flush(2)
                    flush()
                    deferred.append(q_norm_thunk(ssq_half, qc))
                    for t in range(NT):
                        deferred.append(bc_thunk(t, qc))

                # k projection (psum->sbuf copies on Pool to spread DVE load);
                # norms + rskT transposes per qc half so the first half's chain
                # overlaps the second half's waves.
                def k_norm_thunk(ssq_half, qc):
                    def run():
                        lssq = lssq_p.tile([8, G], F32, tag="lssq", name=f"lssq_k{qc}")
                        nc.scalar.activation(out=lssq[:], in_=ssq_half[:], func=AF.Ln)
                        nc.scalar.activation(
                            out=rsk[:, qc * 512 : (qc + 1) * 512],
                            in_=lssq[:], func=AF.Exp, scale=-0.5,
                            bias=lnls[:, 0:1],
                        )

                    return run

                def rskT_thunk(s):
                    def run():
                        pt = pbc.tile([128, G], F32, tag="bc", name=f"rskt{s}")
                        nc.tensor.transpose(
                            pt[:].bitcast(F32R)[:, 0:8],
                            rsk[:, s * 128 : (s + 1) * 128],
                            ident[0:8, 0:8],
                        )
                        nc.gpsimd.tensor_copy(out=rskT[:, s, :], in_=pt[:, 0:8])

                    return run

                for qc in range(QC):
                    ssq_half = pssq_p.tile([8, G], F32, tag="ssq", name=f"ssq_k{qc}")
                    for tp in range(2):
                        proj_wave(wk_sb, kch, kT, bk_sb, ssq_half, nc.gpsimd, qc, tp)
                        flush(2)
                    flush()
                    deferred.append(k_norm_thunk(ssq_half, qc))
                    for s in range(4 * qc, 4 * qc + 4):
                        deferred.append(rskT_thunk(s))
                flush()

            # ---- phase 2: attention + out-projection ----
            with (
                tc.tile_pool(name="psT", bufs=2, space="PSUM") as psT_p,
                tc.tile_pool(name="pv", bufs=1, space="PSUM") as pv_p,
                tc.tile_pool(name="po", bufs=2, space="PSUM") as po_p,
            ):
                ets = {}
                cur_pv = {}
                cur_x = [None] * 8
                vp_issued = [False] * ST
                fill_hi = deque()
                fill_lo = deque()

                def sc(h, s):
                    t, j = divmod(h, 2)
                    st = psT_p.tile([128, N], F32, tag="sT", name=f"sT{h}_{s}")
                    for qc in range(QC):
                        nc.tensor.matmul(
                            st[:, qc * 512 : (qc + 1) * 512],
                            kT[t][j * 64 : (j + 1) * 64, s * 128 : (s + 1) * 128],
                            qT[t][j * 64 : (j + 1) * 64, qc * 512 : (qc + 1) * 512],
                            start=True,
                            stop=True,
                        )
                    e = eT_p.tile([128, N], F16, tag="eT", name=f"eT{h}_{s}")
                    nc.scalar.activation(
                        out=e[:], in_=st[:], func=AF.Exp,
                        bias=lsbias[:, h : h + 1],
                        scale=rskT[:, s, h : h + 1],
                    )
                    ets[(h, s)] = e

                def vp_wave(s):
                    ps = po_p.tile([128, G], F32, tag="po", name=f"vp{s}")
                    for cc in range(CC):
                        nc.tensor.matmul(
                            ps[:],
                            vch[cc][:, s * 128 : (s + 1) * 128],
                            wv_sb[:, cc, :],
                            start=(cc == 0),
                            stop=False,
                        )
                    nc.tensor.matmul(ps[:], ones1[:], bv_sb[:], start=False, stop=True)
                    nc.gpsimd.tensor_copy(
                        out=v_sb[s][:, :, 0:HD],
                        in_=ps[:].rearrange("p (h d) -> p h d", h=8),
                    )
                    vp_issued[s] = True

                def pv_op(h, s):
                    pvA, pvB = cur_pv[h]
                    e = ets.pop((h, s))
                    for qb in range(8):
                        grp = pvA if qb < 4 else pvB
                        nc.tensor.matmul(
                            grp[:, qb % 4, :],
                            e[:, qb * 128 : (qb + 1) * 128],
                            v_sb[s][:, h, :],
                            start=(s == 0),
                            stop=(s == ST - 1),
                        )

                def transp(t, qb, xx):
                    pt = po_p.tile([128, G], F32, tag="po", name=f"tp{t}_{qb}")
                    nc.tensor.transpose(
                        pt[:].bitcast(F32R)[:, 0:128], xx[:], ident[:]
                    )
                    eng = nc.vector if qb % 2 else nc.gpsimd
                    eng.tensor_copy(
                        out=xt[t][:, qb * 128 : (qb + 1) * 128], in_=pt[:, 0:128]
                    )

                def out_pass(s_, coc, ts, first):
                    ps = po_p.tile([128, G], F32, tag="po", name=f"op{s_}_{coc}")
                    for i, t_ in enumerate(ts):
                        nc.tensor.matmul(
                            ps[:],
                            xt[t_][:, s_ * 128 : (s_ + 1) * 128],
                            wo_sb[:, t_, coc * 512 : (coc + 1) * 512],
                            start=(i == 0),
                            stop=(i == len(ts) - 1),
                        )
                    eng = nc.vector if (s_ + coc) % 2 == 0 else nc.gpsimd
                    if first:
                        eng.tensor_copy(out=oA[:, s_, coc, :], in_=ps[:])
                    else:
                        eng.tensor_add(
                            out=oA[:, s_, coc, :], in0=ps[:], in1=oA[:, s_, coc, :]
                        )
                        nc.sync.dma_start(
                            out=out_d[:][
                                s_ * 128 : (s_ + 1) * 128, coc * 512 : (coc + 1) * 512
                            ],
                            in_=oA[:, s_, coc, :],
                        )

                def head_end(h):
                    t, j = divmod(h, 2)
                    pvA, pvB = cur_pv.pop(h)
                    den = den_p.tile([128, 8], F32, tag="den", name=f"den{h}")
                    nc.vector.tensor_copy(out=den[:, 0:4], in_=pvA[:, :, HD])
                    nc.vector.tensor_copy(out=den[:, 4:8], in_=pvB[:, :, HD])
                    rden = den_p.tile([128, 8], F32, tag="rden", name=f"rden{h}")
                    nc.vector.reciprocal(out=rden[:], in_=den[:])
                    for qb in range(8):
                        if j == 0:
                            cur_x[qb] = x_p.tile(
                                [128, 128], F32R, tag=f"x{qb}", name=f"x{qb}_{t}"
                            )
                        grp = pvA if qb < 4 else pvB
                        eng = nc.vector if qb < 4 else nc.gpsimd
                        eng.tensor_scalar_mul(
                            out=cur_x[qb][:, j * 64 : (j + 1) * 64],
                            in0=grp[:, qb % 4, 0:HD],
                            scalar1=rden[:, qb : qb + 1],
                        )
                    if j == 1:
                        for qb in range(8):
                            fill_hi.append(
                                lambda t=t, qb=qb, xx=cur_x[qb]: transp(t, qb, xx)
                            )
                        if t == 1:
                            for s_ in range(ST):
                                for coc in range(2):
                                    fill_lo.append(
                                        lambda s_=s_, coc=coc: out_pass(
                                            s_, coc, (0, 1), True
                                        )
                                    )

                # v-proj waves are scheduled at fixed steps across h0..h2
                vp_sched = {3 * i: i for i in range(ST)}
                pv_ptr = 0

                def pv_ready(ptr, g):
                    h_, s_ = divmod(ptr, 8)
                    if h_ == 0 and not vp_issued[s_]:
                        return False
                    lag = 4 if s_ == 0 else 2
                    return g >= 8 * h_ + s_ + lag if h_ > 0 else g >= s_ + lag

                def drain_pv(g, budget=3):
                    nonlocal pv_ptr
                    while budget > 0 and pv_ptr < 64 and pv_ready(pv_ptr, g):
                        h_, s_ = divmod(pv_ptr, 8)
                        if s_ == 0:
                            cur_pv[h_] = (
                                pv_p.tile([128, 4, HD + 1], F32, tag="pvA", name=f"pvA{h_}"),
                                pv_p.tile([128, 4, HD + 1], F32, tag="pvB", name=f"pvB{h_}"),
                            )
                        pv_op(h_, s_)
                        pv_ptr += 1
                        if s_ == ST - 1:
                            head_end(h_)
                        budget -= 1

                for g in range(64):
                    h, s = divmod(g, 8)
                    sc(h, s)
                    if g in vp_sched:
                        vp_wave(vp_sched[g])
                    elif fill_hi:
                        fill_hi.popleft()()
                    elif fill_lo:
                        fill_lo.popleft()()
                    drain_pv(g)

                # drain: remaining pv ops, last pair's transposes, out-proj B
                while pv_ptr < 64:
                    drain_pv(99, budget=8)
                while fill_hi:
                    fill_hi.popleft()()
                while fill_lo:
                    fill_lo.popleft()()
                for s_ in range(ST):
                    stile = psT_p.tile([128, N], F32, tag="sT", name=f"ob{s_}")
                    for coc in range(2):
                        for i, t_ in enumerate((2, 3)):
                            nc.tensor.matmul(
                                stile[:, coc * 512 : (coc + 1) * 512],
                                xt[t_][:, s_ * 128 : (s_ + 1) * 128],
                                wo_sb[:, t_, coc * 512 : (coc + 1) * 512],
                                start=(i == 0),
                                stop=(i == 1),
                            )
                    for coc in range(2):
                        eng = nc.vector if coc == 0 else nc.gpsimd
                        eng.tensor_add(
                            out=oA[:, s_, coc, :],
                            in0=stile[:, coc * 512 : (coc + 1) * 512],
                            in1=oA[:, s_, coc, :],
                        )
                        nc.sync.dma_start(
                            out=out_d[:][
                                s_ * 128 : (s_ + 1) * 128, coc * 512 : (coc + 1) * 512
                            ],
                            in_=oA[:, s_, coc, :],
                        )

    nc.compile()
    return nc


def kernel(
    query, key, value, in_proj_w, in_proj_b, logit_scale, out_w, out_b, **kw
):
    global _CACHED_NC, _LAST_IN_MAPS
    query = np.asarray(query, dtype=np.float32)
    key = np.asarray(key, dtype=np.float32)
    value = np.asarray(value, dtype=np.float32)
    in_proj_w = np.asarray(in_proj_w, dtype=np.float32)
    in_proj_b = np.asarray(in_proj_b, dtype=np.float32)
    logit_scale = np.asarray(logit_scale, dtype=np.float32)
    out_w = np.asarray(out_w, dtype=np.float32)
    out_b = np.asarray(out_b, dtype=np.float32)

    ls = np.exp(np.minimum(logit_scale.reshape(H), LOGIT_SCALE_MAX))  # (16,)

    # selector constants
    sel8 = np.zeros((NT, 128, 8), dtype=np.float16)
    sel8T = np.zeros((8, NT, 128), dtype=np.float32)
    for t in range(NT):
        for p in range(128):
            h = 2 * t + p // 64
            sel8[t, p, h] = 1.0
            sel8T[h, t, p] = 1.0
    ident = np.eye(128, dtype=np.float32)

    in_maps = []
    for c in range(8):
        b, g = c // 2, c % 2
        dims = slice(g * G, (g + 1) * G)
        ls_c = ls[g * 8 : (g + 1) * 8]  # (8,)
        qt = np.ascontiguousarray(query[:, b, :].T, dtype=np.float16)
        kt = np.ascontiguousarray(key[:, b, :].T, dtype=np.float16)
        vt = np.ascontiguousarray(value[:, b, :].T, dtype=np.float16)
        wq = np.ascontiguousarray(in_proj_w[0 * C :, :][dims, :].T, dtype=np.float16)
        wk = np.ascontiguousarray(in_proj_w[1 * C :, :][dims, :].T, dtype=np.float16)
        wv = np.ascontiguousarray(in_proj_w[2 * C :, :][dims, :].T, dtype=np.float16)
        wo = np.ascontiguousarray(out_w[:, dims].T, dtype=np.float16)
        bq = np.ascontiguousarray(in_proj_b[0 * C :][dims].reshape(NT, 128).T)
        bk = np.ascontiguousarray(in_proj_b[1 * C :][dims].reshape(NT, 128).T)
        bv = np.ascontiguousarray(
            in_proj_b[2 * C :][dims].reshape(1, G), dtype=np.float16
        )
        lnls = np.log(ls_c).reshape(8, 1).astype(np.float32)
        lsbias = np.repeat(-ls_c.reshape(1, 8), 128, axis=0).astype(np.float32)
        in_maps.append(
            {
                "qt": qt,
                "kt": kt,
                "vt": vt,
                "wq": wq,
                "wk": wk,
                "wv": wv,
                "wo": wo,
                "bq": bq.astype(np.float32),
                "bk": bk.astype(np.float32),
                "bv": bv,
                "sel8": sel8,
                "sel8T": sel8T,
                "lnls": lnls,
                "lsbias": lsbias,
                "ident": ident,
                "ones1": np.ones((1, 128), dtype=np.float16),
            }
        )

    _LAST_IN_MAPS = in_maps
    if _CACHED_NC is None:
        _CACHED_NC = build_nc()
    res = run_bass_kernel_spmd(_CACHED_NC, in_maps, core_ids=list(range(8)))

    out = np.zeros((N, B, C), dtype=np.float32)
    for c in range(8):
        b = c // 2
        out[:, b, :] += res.results[c]["out"]
    out += out_b.reshape(1, 1, C)
    return out


# revision 6
# speedup vs baseline: 1.4344x; 1.0138x over previous
# Trainium2 Bass kernel for nn_CustomAttention (cosine-sim multi-head attention).
#
# Sharding over 8 cores: core c handles batch b = c//2 and head group
# g = c%2 (8 of 16 heads, 512 feature dims).  Each core computes its heads'
# q/k/v projections (Megatron column-parallel), cosine-sim attention, and a
# partial output projection (row-parallel over its 512 dims).  The host sums
# the two partial outputs per batch and adds out_b.
#
# All heavy matmuls run in fp16 (1 cycle/row on the PE at any output width,
# unlike f32r which needs >=256 moving columns), with fp32 PSUM accumulation.
# Layout highlights:
#   qT/kT: (dims=512, seq=1024) as 4 tiles of (128, 1024); head dims on
#          partitions so the scores matmul contracts head_dim on partitions.
#   scores are computed transposed, sT[k, q]; the per-key 1/||k||*ls factor
#   and the -ls bias fold into the exp() activation as per-partition APs.
#   1/||q|| and 1/||k|| come from exp(-0.5*ln(ssq)) so the whole kernel uses
#   a single activation table (ln+exp) -- no Sqrt table reloads.
#   p@v runs transposed: out[x: q, d] = eT[k, q-block]^T @ v[k, d|1]; the
#   appended ones-column of v gives the softmax denominator, which is applied
#   as a per-partition (per-query) scalar during the PSUM->SBUF copy.
#   x tiles are then PE-transposed back to (dims, seq) for the out-proj.
#   out-proj is split into two half-contractions (t0+t1, t2+t3) so most of it
#   overlaps the attention stream; the halves are summed in SBUF.
#
# The attention main loop software-pipelines scores/exp (ACT-paced) against
# p@v, the v projection, x transposes and the first out-proj half, which are
# woven into the PE instruction stream as fillers (engine queues drain
# strictly in issue order, so issue order == execution order).

import math
import sys
from collections import deque

import numpy as np

sys.path.insert(0, "/opt/trn_rl_repo")

import concourse.bass as bass
import concourse.tile as tile
from concourse import bacc, mybir
from concourse.bass_utils import run_bass_kernel_spmd

N = 1024  # sequence length
B = 4  # batch
C = 1024  # channels
H = 16  # total heads
HD = 64  # head dim
G = 512  # dims per core (8 heads)
NT = 4  # (128, N) tiles of qT/kT per core
CC = 8  # contraction chunks of 128 over C
ST = 8  # seq tiles of 128
QC = 2  # seq chunks of 512
LOGIT_SCALE_MAX = math.log(1.0 / 0.01)

F32 = mybir.dt.float32
F32R = mybir.dt.float32r
F16 = mybir.dt.float16
AF = mybir.ActivationFunctionType

_CACHED_NC = None
_LAST_IN_MAPS = None


def build_nc():
    nc = bacc.Bacc("TRN2", target_bir_lowering=False)

    qt_d = nc.declare_dram_parameter("qt", [C, N], F16, isOutput=False)
    kt_d = nc.declare_dram_parameter("kt", [C, N], F16, isOutput=False)
    vt_d = nc.declare_dram_parameter("vt", [C, N], F16, isOutput=False)
    wq_d = nc.declare_dram_parameter("wq", [C, G], F16, isOutput=False)
    wk_d = nc.declare_dram_parameter("wk", [C, G], F16, isOutput=False)
    wv_d = nc.declare_dram_parameter("wv", [C, G], F16, isOutput=False)
    wo_d = nc.declare_dram_parameter("wo", [G, C], F16, isOutput=False)
    bq_d = nc.declare_dram_parameter("bq", [128, NT], F32, isOutput=False)
    bk_d = nc.declare_dram_parameter("bk", [128, NT], F32, isOutput=False)
    bv_d = nc.declare_dram_parameter("bv", [1, G], F16, isOutput=False)
    sel8_d = nc.declare_dram_parameter("sel8", [NT, 128, 8], F16, isOutput=False)
    sel8T_d = nc.declare_dram_parameter("sel8T", [8, NT, 128], F32R, isOutput=False)
    lnls_d = nc.declare_dram_parameter("lnls", [8, 1], F32, isOutput=False)
    lsbias_d = nc.declare_dram_parameter("lsbias", [128, 8], F32, isOutput=False)
    ident_d = nc.declare_dram_parameter("ident", [128, 128], F32R, isOutput=False)
    ones1_d = nc.declare_dram_parameter("ones1", [1, 128], F16, isOutput=False)
    out_d = nc.declare_dram_parameter("out", [N, C], F16, isOutput=True)

    qt_r = qt_d[:].rearrange("(cc p) n -> cc p n", p=128)
    kt_r = kt_d[:].rearrange("(cc p) n -> cc p n", p=128)
    vt_r = vt_d[:].rearrange("(cc p) n -> cc p n", p=128)
    # weight halves: [2, 128, 4, G]
    wq_r = wq_d[:].rearrange("(g cc p) o -> g p cc o", g=2, p=128)
    wk_r = wk_d[:].rearrange("(g cc p) o -> g p cc o", g=2, p=128)
    wv_r = wv_d[:].rearrange("(g cc p) o -> g p cc o", g=2, p=128)

    # pre-load the ln+exp activation table once; every activation in this
    # kernel (Ln, Exp) is servable from it, so the auto-inserted loads
    # (which thrash between exp-only and ln-only tables) are avoided.
    from concourse.hw_specs import get_activation_tables

    table_names = list(get_activation_tables(nc.m.arch).keys())
    lnexp_id = table_names.index("natural_log_exp_and_others")

    with tile.TileContext(nc) as tc:
        nc.scalar.add_instruction(
            mybir.InstLoadActFuncSet(
                name=nc.get_next_instruction_name(), ins=[], outs=[],
                act_func_set_id=lnexp_id,
            )
        )
        with (
            tc.tile_pool(name="consts", bufs=1) as consts,
            tc.tile_pool(name="wo_p", bufs=1) as wo_p,
            tc.tile_pool(name="w_p", bufs=3) as w_p,
            tc.tile_pool(name="acts", bufs=18) as acts,
            tc.tile_pool(name="big", bufs=1) as big,
            tc.tile_pool(name="sq_p", bufs=2) as sq_p,
            tc.tile_pool(name="stats", bufs=1) as stats,
            tc.tile_pool(name="lssq_p", bufs=2) as lssq_p,
            tc.tile_pool(name="eT_p", bufs=18) as eT_p,
            tc.tile_pool(name="x_p", bufs=2) as x_p,
            tc.tile_pool(name="den_p", bufs=2) as den_p,
            tc.tile_pool(name="oA_p", bufs=1) as oA_p,
        ):
            # ---- persistent tiles ----
            qT = [big.tile([128, N], F16, tag=f"qT{t}", name=f"qT{t}") for t in range(NT)]
            kT = [big.tile([128, N], F16, tag=f"kT{t}", name=f"kT{t}") for t in range(NT)]
            v_sb = [big.tile([128, 8, HD + 1], F16, tag=f"v{s}", name=f"v{s}") for s in range(ST)]
            xt = [big.tile([128, N], F16, tag=f"xt{t}", name=f"xt{t}") for t in range(NT)]
            rskT = stats.tile([128, ST, 8], F32)
            rsq = stats.tile([8, N], F32R)
            rsk = stats.tile([8, N], F32R)
            oA = oA_p.tile([128, ST, 2, G], F16)

            # ones column of v (softmax denominator); disjoint from the
            # projection writes so it can be set up front.
            for s in range(ST):
                nc.gpsimd.memset(v_sb[s][:, :, HD], 1.0)

            # ---- DMA stream (single SP queue; issue order = transfer order) ----
            def stream_acts(dram_r, nm):
                ch = []
                for cc in range(CC):
                    a = acts.tile([128, N], F16, tag="act", name=f"{nm}{cc}")
                    nc.sync.dma_start(out=a[:], in_=dram_r[cc])
                    ch.append(a)
                return ch

            # q stream: first act chunks interleaved with the weight halves
            wq_sb = w_p.tile([128, CC, G], F16, tag="w", name="wq")
            qch = []
            a0 = acts.tile([128, N], F16, tag="act", name="qt0")
            nc.sync.dma_start(out=a0[:], in_=qt_r[0])
            qch.append(a0)
            nc.sync.dma_start(out=wq_sb[:, 0:4, :], in_=wq_r[0])
            for cc in range(1, 4):
                a = acts.tile([128, N], F16, tag="act", name=f"qt{cc}")
                nc.sync.dma_start(out=a[:], in_=qt_r[cc])
                qch.append(a)
            nc.sync.dma_start(out=wq_sb[:, 4:8, :], in_=wq_r[1])
            for cc in range(4, 8):
                a = acts.tile([128, N], F16, tag="act", name=f"qt{cc}")
                nc.sync.dma_start(out=a[:], in_=qt_r[cc])
                qch.append(a)

            # consts needed during the q projection
            sel8 = consts.tile([128, NT, 8], F16)
            nc.sync.dma_start(out=sel8[:], in_=sel8_d[:].rearrange("t p e -> p t e"))
            bq_sb = consts.tile([128, NT], F32)
            nc.sync.dma_start(out=bq_sb[:], in_=bq_d[:])

            wk_sb = w_p.tile([128, CC, G], F16, tag="w", name="wk")
            nc.sync.dma_start(out=wk_sb[:, 0:4, :], in_=wk_r[0])
            nc.sync.dma_start(out=wk_sb[:, 4:8, :], in_=wk_r[1])
            kch = stream_acts(kt_r, "kt")

            sel8T = consts.tile([8, NT, 128], F32R)
            nc.sync.dma_start(out=sel8T[:], in_=sel8T_d[:])
            bk_sb = consts.tile([128, NT], F32)
            nc.sync.dma_start(out=bk_sb[:], in_=bk_d[:])
            lnls = consts.tile([8, 1], F32)
            nc.sync.dma_start(out=lnls[:], in_=lnls_d[:])
            ident = consts.tile([128, 128], F32R)
            nc.sync.dma_start(out=ident[:], in_=ident_d[:])

            wv_sb = w_p.tile([128, CC, G], F16, tag="w", name="wv")
            nc.sync.dma_start(out=wv_sb[:, 0:4, :], in_=wv_r[0])
            nc.sync.dma_start(out=wv_sb[:, 4:8, :], in_=wv_r[1])
            vch = stream_acts(vt_r, "vt")

            lsbias = consts.tile([128, 8], F32)
            nc.sync.dma_start(out=lsbias[:], in_=lsbias_d[:])
            ones1 = consts.tile([1, 128], F16)
            nc.sync.dma_start(out=ones1[:], in_=ones1_d[:])
            bv_sb = consts.tile([1, G], F16)
            nc.sync.dma_start(out=bv_sb[:], in_=bv_d[:])
            wo_sb = wo_p.tile([128, NT, C], F16)
            nc.sync.dma_start(
                out=wo_sb[:], in_=wo_d[:].rearrange("(t p) c -> p t c", p=128)
            )

            # ---- phase 1: q/k projections + norms ----
            deferred = deque()

            def flush(n=99):
                for _ in range(min(n, len(deferred))):
                    deferred.popleft()()

            with (
                tc.tile_pool(name="pp", bufs=4, space="PSUM") as pp,
                tc.tile_pool(name="pssq", bufs=2, space="PSUM") as pssq_p,
                tc.tile_pool(name="pbc", bufs=2, space="PSUM") as pbc,
            ):
                def proj_wave(w_sb, ch, dst, b_sb, ssq_half, copy_eng, qc, tp):
                    pst = [pp.tile([128, G], F32, tag="proj", name=f"proj{qc}{tp}{i}") for i in range(2)]
                    for cc in range(CC):
                        for ti in range(2):
                            t = 2 * tp + ti
                            nc.tensor.matmul(
                                pst[ti][:],
                                w_sb[:, cc, t * 128 : (t + 1) * 128],
                                ch[cc][:, qc * 512 : (qc + 1) * 512],
                                start=(cc == 0),
                                stop=(cc == CC - 1),
                            )
                    for ti in range(2):
                        t = 2 * tp + ti
                        copy_eng.tensor_scalar_add(
                            out=dst[t][:, qc * 512 : (qc + 1) * 512],
                            in0=pst[ti][:],
                            scalar1=b_sb[:, t : t + 1],
                        )
                        sq = sq_p.tile([128, G], F16, tag="sq", name=f"sq{t}{qc}")
                        nc.vector.tensor_mul(
                            out=sq[:],
                            in0=dst[t][:, qc * 512 : (qc + 1) * 512],
                            in1=dst[t][:, qc * 512 : (qc + 1) * 512],
                        )

                        def ssq_mm(t=t, sq=sq):
                            nc.tensor.matmul(
                                ssq_half[:],
                                sel8[:, t, :],
                                sq[:],
                                start=(t == 0),
                                stop=(t == NT - 1),
                            )

                        deferred.append(ssq_mm)

                # q projection; norms per qc half: rsq = exp(-0.5*ln(ssq))
                def q_norm_thunk(ssq_half, qc):
                    def run():
                        lssq = lssq_p.tile([8, G], F32, tag="lssq", name=f"lssq_q{qc}")
                        nc.scalar.activation(out=lssq[:], in_=ssq_half[:], func=AF.Ln)
                        nc.scalar.activation(
                            out=rsq[:, qc * 512 : (qc + 1) * 512],
                            in_=lssq[:], func=AF.Exp, scale=-0.5,
                        )

                    return run

                def bc_thunk(t, qc):
                    def run():
                        pb = pbc.tile([128, G], F32, tag="bc", name=f"bc{t}{qc}")
                        nc.tensor.matmul(
                            pb[:],
                            sel8T[:, t, :],
                            rsq[:, qc * 512 : (qc + 1) * 512],
                            start=True,
                            stop=True,
                        )
                        nc.vector.tensor_mul(
                            out=qT[t][:, qc * 512 : (qc + 1) * 512],
                            in0=qT[t][:, qc * 512 : (qc + 1) * 512],
                            in1=pb[:],
                        )

                    return run

                for qc in range(QC):
                    ssq_half = pssq_p.tile([8, G], F32, tag="ssq", name=f"ssq_q{qc}")
                    for tp in range(2):
                        proj_wave(wq_sb, qch, qT, bq_sb, ssq_half, nc.vector, qc, tp)
                        flush(2)
                    flush()
                    deferred.append(q_norm_thunk(ssq_half, qc))
                    for t in range(NT):
                        deferred.append(bc_thunk(t, qc))

                # k projection (psum->sbuf copies on Pool to spread DVE load);
                # norms + rskT transposes per qc half so the first half's chain
                # overlaps the second half's waves.
                def k_norm_thunk(ssq_half, qc):
                    def run():
                        lssq = lssq_p.tile([8, G], F32, tag="lssq", name=f"lssq_k{qc}")
                        nc.scalar.activation(out=lssq[:], in_=ssq_half[:], func=AF.Ln)
                        nc.scalar.activation(
                            out=rsk[:, qc * 512 : (qc + 1) * 512],
                            in_=lssq[:], func=AF.Exp, scale=-0.5,
                            bias=lnls[:, 0:1],
                        )

                    return run

                def rskT_thunk(s):
                    def run():
                        pt = pbc.tile([128, G], F32, tag="bc", name=f"rskt{s}")
                        nc.tensor.transpose(
                            pt[:].bitcast(F32R)[:, 0:8],
                            rsk[:, s * 128 : (s + 1) * 128],
                            ident[0:8, 0:8],
                        )
                        nc.gpsimd.tensor_copy(out=rskT[:, s, :], in_=pt[:, 0:8])

                    return run

                for qc in range(QC):
                    ssq_half = pssq_p.tile([8, G], F32, tag="ssq", name=f"ssq_k{qc}")
                    for tp in range(2):
                        proj_wave(wk_sb, kch, kT, bk_sb, ssq_half, nc.gpsimd, qc, tp)
                        flush(2)
                    flush()
                    deferred.append(k_norm_thunk(ssq_half, qc))
                    for s in range(4 * qc, 4 * qc + 4):
                        deferred.append(rskT_thunk(s))
                flush()

            # ---- phase 2: attention + out-projection ----
            with (
                tc.tile_pool(name="psT", bufs=2, space="PSUM") as psT_p,
                tc.tile_pool(name="pv", bufs=1, space="PSUM") as pv_p,
                tc.tile_pool(name="po", bufs=2, space="PSUM") as po_p,
            ):
                ets = {}
                cur_pv = {}
                cur_x = [None] * 8
                vp_issued = [False] * ST
                fill_hi = deque()
                fill_lo = deque()

                def sc(h, s):
                    t, j = divmod(h, 2)
                    st = psT_p.tile([128, N], F32, tag="sT", name=f"sT{h}_{s}")
                    for qc in range(QC):
                        nc.tensor.matmul(
                            st[:, qc * 512 : (qc + 1) * 512],
                            kT[t][j * 64 : (j + 1) * 64, s * 128 : (s + 1) * 128],
                            qT[t][j * 64 : (j + 1) * 64, qc * 512 : (qc + 1) * 512],
                            start=True,
                            stop=True,
                        )
                    e = eT_p.tile([128, N], F16, tag="eT", name=f"eT{h}_{s}")
                    nc.scalar.activation(
                        out=e[:], in_=st[:], func=AF.Exp,
                        bias=lsbias[:, h : h + 1],
                        scale=rskT[:, s, h : h + 1],
                    )
                    ets[(h, s)] = e

                def vp_wave(s):
                    ps = po_p.tile([128, G], F32, tag="po", name=f"vp{s}")
                    for cc in range(CC):
                        nc.tensor.matmul(
                            ps[:],
                            vch[cc][:, s * 128 : (s + 1) * 128],
                            wv_sb[:, cc, :],
                            start=(cc == 0),
                            stop=False,
                        )
                    nc.tensor.matmul(ps[:], ones1[:], bv_sb[:], start=False, stop=True)
                    nc.gpsimd.tensor_copy(
                        out=v_sb[s][:, :, 0:HD],
                        in_=ps[:].rearrange("p (h d) -> p h d", h=8),
                    )
                    vp_issued[s] = True

                def pv_op(h, s):
                    pvA, pvB = cur_pv[h]
                    e = ets.pop((h, s))
                    for qb in range(8):
                        grp = pvA if qb < 4 else pvB
                        nc.tensor.matmul(
                            grp[:, qb % 4, :],
                            e[:, qb * 128 : (qb + 1) * 128],
                            v_sb[s][:, h, :],
                            start=(s == 0),
                            stop=(s == ST - 1),
                        )

                def transp(t, qb, xx):
                    pt = po_p.tile([128, G], F32, tag="po", name=f"tp{t}_{qb}")
                    nc.tensor.transpose(
                        pt[:].bitcast(F32R)[:, 0:128], xx[:], ident[:]
                    )
                    eng = nc.vector if qb % 2 else nc.gpsimd
                    eng.tensor_copy(
                        out=xt[t][:, qb * 128 : (qb + 1) * 128], in_=pt[:, 0:128]
                    )

                def out_pass(s_, coc, ts, first):
                    ps = po_p.tile([128, G], F32, tag="po", name=f"op{s_}_{coc}")
                    for i, t_ in enumerate(ts):
                        nc.tensor.matmul(
                            ps[:],
                            xt[t_][:, s_ * 128 : (s_ + 1) * 128],
                            wo_sb[:, t_, coc * 512 : (coc + 1) * 512],
                            start=(i == 0),
                            stop=(i == len(ts) - 1),
                        )
                    eng = nc.vector if (s_ + coc) % 2 == 0 else nc.gpsimd
                    if first:
                        eng.tensor_copy(out=oA[:, s_, coc, :], in_=ps[:])
                    else:
                        eng.tensor_add(
                            out=oA[:, s_, coc, :], in0=ps[:], in1=oA[:, s_, coc, :]
                        )
                        nc.sync.dma_start(
                            out=out_d[:][
                                s_ * 128 : (s_ + 1) * 128, coc * 512 : (coc + 1) * 512
                            ],
                            in_=oA[:, s_, coc, :],
                        )

                def head_end(h):
                    t, j = divmod(h, 2)
                    pvA, pvB = cur_pv.pop(h)
                    den = den_p.tile([128, 8], F32, tag="den", name=f"den{h}")
                    nc.vector.tensor_copy(out=den[:, 0:4], in_=pvA[:, :, HD])
                    nc.vector.tensor_copy(out=den[:, 4:8], in_=pvB[:, :, HD])
                    rden = den_p.tile([128, 8], F32, tag="rden", name=f"rden{h}")
                    nc.vector.reciprocal(out=rden[:], in_=den[:])
                    for qb in range(8):
                        if j == 0:
                            cur_x[qb] = x_p.tile(
                                [128, 128], F32R, tag=f"x{qb}", name=f"x{qb}_{t}"
                            )
                        grp = pvA if qb < 4 else pvB
                        eng = nc.vector if qb < 4 else nc.gpsimd
                        eng.tensor_scalar_mul(
                            out=cur_x[qb][:, j * 64 : (j + 1) * 64],
                            in0=grp[:, qb % 4, 0:HD],
                            scalar1=rden[:, qb : qb + 1],
                        )
                    if j == 1:
                        for qb in range(8):
                            fill_hi.append(
                                lambda t=t, qb=qb, xx=cur_x[qb]: transp(t, qb, xx)
                            )
                        if t == 1:
                            for s_ in range(ST):
                                for coc in range(2):
                                    fill_lo.append(
                                        lambda s_=s_, coc=coc: out_pass(
                                            s_, coc, (0, 1), True
                                        )
                                    )

                # v-proj waves are scheduled at fixed steps across h0..h2
                vp_sched = {3 * i: i for i in range(ST)}
                pv_ptr = 0

                def pv_ready(ptr, g):
                    h_, s_ = divmod(ptr, 8)
                    if h_ == 0 and not vp_issued[s_]:
                        return False
                    lag = 4 if s_ == 0 else 2
                    return g >= 8 * h_ + s_ + lag if h_ > 0 else g >= s_ + lag

                def drain_pv(g, budget=2):
                    nonlocal pv_ptr
                    while budget > 0 and pv_ptr < 64 and pv_ready(pv_ptr, g):
                        h_, s_ = divmod(pv_ptr, 8)
                        if s_ == 0:
                            cur_pv[h_] = (
                                pv_p.tile([128, 4, HD + 1], F32, tag="pvA", name=f"pvA{h_}"),
                                pv_p.tile([128, 4, HD + 1], F32, tag="pvB", name=f"pvB{h_}"),
                            )
                        pv_op(h_, s_)
                        pv_ptr += 1
                        if s_ == ST - 1:
                            head_end(h_)
                        budget -= 1

                for g in range(64):
                    h, s = divmod(g, 8)
                    sc(h, s)
                    if g in vp_sched:
                        vp_wave(vp_sched[g])
                    elif fill_hi:
                        fill_hi.popleft()()
                    elif fill_lo:
                        fill_lo.popleft()()
                    drain_pv(g)

                # drain: remaining pv ops, last pair's transposes, out-proj B
                while pv_ptr < 64:
                    drain_pv(99, budget=8)
                while fill_lo:
                    fill_lo.popleft()()
                for s_ in range(ST):
                    if fill_hi:
                        fill_hi.popleft()()
                    stile = psT_p.tile([128, N], F32, tag="sT", name=f"ob{s_}")
                    for coc in range(2):
                        for i, t_ in enumerate((2, 3)):
                            nc.tensor.matmul(
                                stile[:, coc * 512 : (coc + 1) * 512],
                                xt[t_][:, s_ * 128 : (s_ + 1) * 128],
                                wo_sb[:, t_, coc * 512 : (coc + 1) * 512],
                                start=(i == 0),
                                stop=(i == 1),
                            )
                    for coc in range(2):
                        eng = nc.vector if coc == 0 else nc.gpsimd
                        eng.tensor_add(
                            out=oA[:, s_, coc, :],
                            in0=stile[:, coc * 512 : (coc + 1) * 512],
                            in1=oA[:, s_, coc, :],
                        )
                        nc.sync.dma_start(
                            out=out_d[:][
                                s_ * 128 : (s_ + 1) * 128, coc * 512 : (coc + 1) * 512
                            ],
                            in_=oA[:, s_, coc, :],
                        )

    nc.compile()
    return nc


def kernel(
    query, key, value, in_proj_w, in_proj_b, logit_scale, out_w, out_b, **kw
):
    global _CACHED_NC, _LAST_IN_MAPS
    query = np.asarray(query, dtype=np.float32)
    key = np.asarray(key, dtype=np.float32)
    value = np.asarray(value, dtype=np.float32)
    in_proj_w = np.asarray(in_proj_w, dtype=np.float32)
    in_proj_b = np.asarray(in_proj_b, dtype=np.float32)
    logit_scale = np.asarray(logit_scale, dtype=np.float32)
    out_w = np.asarray(out_w, dtype=np.float32)
    out_b = np.asarray(out_b, dtype=np.float32)

    ls = np.exp(np.minimum(logit_scale.reshape(H), LOGIT_SCALE_MAX))  # (16,)

    # selector constants
    sel8 = np.zeros((NT, 128, 8), dtype=np.float16)
    sel8T = np.zeros((8, NT, 128), dtype=np.float32)
    for t in range(NT):
        for p in range(128):
            h = 2 * t + p // 64
            sel8[t, p, h] = 1.0
            sel8T[h, t, p] = 1.0
    ident = np.eye(128, dtype=np.float32)

    in_maps = []
    for c in range(8):
        b, g = c // 2, c % 2
        dims = slice(g * G, (g + 1) * G)
        ls_c = ls[g * 8 : (g + 1) * 8]  # (8,)
        qt = np.ascontiguousarray(query[:, b, :].T, dtype=np.float16)
        kt = np.ascontiguousarray(key[:, b, :].T, dtype=np.float16)
        vt = np.ascontiguousarray(value[:, b, :].T, dtype=np.float16)
        wq = np.ascontiguousarray(in_proj_w[0 * C :, :][dims, :].T, dtype=np.float16)
        wk = np.ascontiguousarray(in_proj_w[1 * C :, :][dims, :].T, dtype=np.float16)
        wv = np.ascontiguousarray(in_proj_w[2 * C :, :][dims, :].T, dtype=np.float16)
        wo = np.ascontiguousarray(out_w[:, dims].T, dtype=np.float16)
        bq = np.ascontiguousarray(in_proj_b[0 * C :][dims].reshape(NT, 128).T)
        bk = np.ascontiguousarray(in_proj_b[1 * C :][dims].reshape(NT, 128).T)
        bv = np.ascontiguousarray(
            in_proj_b[2 * C :][dims].reshape(1, G), dtype=np.float16
        )
        lnls = np.log(ls_c).reshape(8, 1).astype(np.float32)
        lsbias = np.repeat(-ls_c.reshape(1, 8), 128, axis=0).astype(np.float32)
        in_maps.append(
            {
                "qt": qt,
                "kt": kt,
                "vt": vt,
                "wq": wq,
                "wk": wk,
                "wv": wv,
                "wo": wo,
                "bq": bq.astype(np.float32),
                "bk": bk.astype(np.float32),
                "bv": bv,
                "sel8": sel8,
                "sel8T": sel8T,
                "lnls": lnls,
                "lsbias": lsbias,
                "ident": ident,
                "ones1": np.ones((1, 128), dtype=np.float16),
            }
        )

    _LAST_IN_MAPS = in_maps
    if _CACHED_NC is None:
        _CACHED_NC = build_nc()
    res = run_bass_kernel_spmd(_CACHED_NC, in_maps, core_ids=list(range(8)))

    out = np.zeros((N, B, C), dtype=np.float32)
    for c in range(8):
        b = c // 2
        out[:, b, :] += res.results[c]["out"].astype(np.float32)
    out += out_b.reshape(1, 1, C)
    return out
